# revision 21
# baseline (speedup 1.0000x reference)
"""Trainium2 Bass kernel for nn_DiT_4758823763997 (DiT dense transformer).

B=8 batch, N=256 tokens, D=768, 12 layers, 12 heads (hd 64), MLP 3072.
Sharding: pure data-parallel - one batch element per NeuronCore (8 cores),
weights replicated; no collectives.

v3 design (vs v2):
  - Per-token-tile (t0/t1) software pipelining across ALL phase boundaries:
    the serial DVE LayerNorm/AdaLN chains for tile t run while the PE works
    on the other tile's GEMMs (prev-layer mlp-down t1, per-t QK, per-t
    mlp-up), eliminating the two ~8us PE stalls per layer.
  - LN rstd computed as exp(-0.5*ln(var+eps)): Ln and Exp share one ACT
    table set, so the only ACT table swaps left are gelu<->ln/exp (2 per
    layer), both prefetched via dummy ops while ACT is idle.
  - Rotary processed 512-wide (q|k merged per chunk) on DVE in bf16.
  - Softmax normalizer batched per 6-head group: AV results accumulate in
    one PSUM bank per (t, half), one strided reciprocal + one stride-0
    broadcast multiply replace 24 reciprocal+scale pairs.
  - Out-proj bias folded into the GEMM accumulation (ones-row matmul).
  - Small latency-critical DMAs (per-layer rows, GN row) issued on the
    scalar HWDGE ring so the weight-prefetch flood on the sync ring cannot
    delay them.
"""

import math
import os
import sys

sys.path.insert(0, "/opt/trn_rl_repo")

import numpy as np

import concourse.bass as bass
import concourse.bacc as bacc
import concourse.mybir as mybir
import concourse.tile as tile
from concourse.bass_utils import run_bass_kernel_spmd

B = 8
C_IN = 3
HH = 256
WW = 256
P = 16
D = 768
DEPTH = 12
NH = 12
HD = 64
MLPD = 3072
N = 256
G = 8
GS = D // G

F32 = mybir.dt.float32
BF16 = mybir.dt.bfloat16
AF = mybir.ActivationFunctionType
OP = mybir.AluOpType

DC = D // 128    # 6
NT = N // 128    # 2
MC = MLPD // 128  # 24
U0 = 8           # mlp-up t0 runahead chunks (even; bounds live PSUM pairs)

LAST_RESULT = {}
_CACHE = {}

# stream_shuffle mask: swap adjacent partitions within each 32-quadrant
SWAP_MASK = [i ^ 1 for i in range(32)]


def _ap3(ap2d, base, nblk, stride, width):
    """[128, nblk, width] free-strided view of a 2D AP at column offset base."""
    return bass.AP(tensor=ap2d.tensor, offset=ap2d.offset + base,
                   ap=[ap2d.ap[0], [stride, nblk], [1, width]])


def _row_bcast(row_ap, width, parts=128):
    """[1, W] row -> step-0 partition-broadcast AP [parts, W]."""
    return bass.AP(tensor=row_ap.tensor, offset=row_ap.offset,
                   ap=[[0, parts], [1, width]])


def _build():
    nc = bacc.Bacc("TRN2", target_bir_lowering=False, debug=False, num_devices=8)

    def din(name, shape, dt=BF16):
        return nc.declare_dram_parameter(name, list(shape), dt, isOutput=False)

    xcolT = din("xcolT", [D, N])
    identm = din("identm", [128, 128])
    onesr = din("onesr", [1, 128])
    convw = din("convw", [D, D])
    convbr = din("convbr", [1, D])
    grow = din("grow", [1, 3 * D + 2 * G], F32)   # gn_g | gn_b | scratch
    cosPP = din("cosPP", [128, 2 * N])
    sinPP = din("sinPP", [128, 2 * N])
    Lw = []
    for i in range(DEPTH):
        Lw.append(dict(
            wqkvo=din(f"wqkvo{i}", [D, 4 * D]),        # wq|wk|wv|wo (q,k col-permuted)
            w1=din(f"w1{i}", [D, MLPD]),
            w2p=din(f"w2p{i}", [128, MC * D]),          # pre-chunked [128, 24*768]
            lrow=din(f"lrow{i}", [1, 5 * D]),           # shift|mod1|bv|b2|bo (bf16)
            smalls=din(f"smalls{i}", [128, 12 + MC], F32),  # bqP|bkP|b1c
        ))
    outw = din("outw", [D, D])
    outrow = din("outrow", [1, D], F32)
    out = nc.declare_dram_parameter("out", [N, D], F32, isOutput=True)

    with tile.TileContext(nc) as tc:
        _emit(nc, tc, xcolT, identm, onesr, convw, convbr, grow, cosPP, sinPP,
              Lw, outw, outrow, out)
    nc.compile()
    return nc


def _emit(nc, tc, xcolT, identm, onesr, convw, convbr, grow, cosPP, sinPP,
          Lw, outw, outrow, out):
    from contextlib import ExitStack
    with ExitStack() as ctx:
        pers = ctx.enter_context(tc.tile_pool(name="pers", bufs=1))
        wp = ctx.enter_context(tc.tile_pool(name="wp", bufs=13))     # [128,3072] bf16 weight tiles
        res = ctx.enter_context(tc.tile_pool(name="res", bufs=6))
        tr = ctx.enter_context(tc.tile_pool(name="tr", bufs=4))
        wt = ctx.enter_context(tc.tile_pool(name="wt", bufs=4))      # transposed activations bf16
        rq = ctx.enter_context(tc.tile_pool(name="rq", bufs=7))      # rotated q|k chunks
        rt = ctx.enter_context(tc.tile_pool(name="rt", bufs=4))      # rotary transients
        st = ctx.enter_context(tc.tile_pool(name="st", bufs=6))
        ex = ctx.enter_context(tc.tile_pool(name="ex", bufs=3))
        ge = ctx.enter_context(tc.tile_pool(name="ge", bufs=26))     # gelu chunks (24 live)
        lc = ctx.enter_context(tc.tile_pool(name="lc", bufs=2))      # bcast rows bf16
        sm = ctx.enter_context(tc.tile_pool(name="sm", bufs=2))      # smalls
        ec = ctx.enter_context(tc.tile_pool(name="ec", bufs=1))
        pp = ctx.enter_context(tc.tile_pool(name="pp", bufs=8, space="PSUM"))

        ident = pers.tile([128, 128], BF16, tag="ident", name="ident")
        ones_col = pers.tile([128, 1], BF16, tag="onesc", name="onesc")
        nc.sync.dma_start(
            out=ones_col[:],
            in_=bass.AP(tensor=onesr[:1, :].tensor, offset=onesr[:1, :].offset,
                        ap=[[0, 128], [1, 1]]))
        ones_row = pers.tile([1, 128], BF16, tag="onesr", name="onesr")
        nc.sync.dma_start(out=ones_row[:], in_=onesr[:1, :])
        eps6 = pers.tile([128, 1], F32, tag="eps6", name="eps6")
        nc.vector.memset(eps6[:], 1e-6)
        eps5 = pers.tile([128, 1], F32, tag="eps5", name="eps5")
        nc.vector.memset(eps5[:], 1e-5)
        dmy = pers.tile([1, 2], F32, tag="dmy", name="dmy")
        nc.vector.memset(dmy[:], 1.0)

        cosPt = pers.tile([128, 2 * N], BF16, tag="cosPP", name="cosPP")
        sinPt = pers.tile([128, 2 * N], BF16, tag="sinPP", name="sinPP")

        h = [pers.tile([128, D], F32, tag=f"h{t}", name=f"h{t}") for t in range(NT)]
        v_aug = [pers.tile([128, NH * 66], BF16, tag=f"va{t}", name=f"va{t}")
                 for t in range(NT)]

        def late_const_dmas():
            # constants not needed until layer 0: emitted after the embed's
            # input DMAs so they don't delay the first conv matmuls
            nc.sync.dma_start(out=ident[:], in_=identm[:, :])
            nc.sync.dma_start(out=cosPt[:], in_=cosPP[:, :])
            nc.sync.dma_start(out=sinPt[:], in_=sinPP[:, :])
            for t in range(NT):
                va = v_aug[t][:]
                nc.sync.dma_start(
                    out=bass.AP(tensor=va.tensor, offset=va.offset + 64,
                                ap=[va.ap[0], [66, NH], [1, 2]]),
                    in_=bass.AP(tensor=onesr[:1, :].tensor,
                                offset=onesr[:1, :].offset,
                                ap=[[0, 128], [1, 2 * NH]]))

        def ln_apply(x_ap, out_ap):
            """out = (x - mean)/sqrt(var + 1e-6) along free dim 768."""
            s = st.tile([128, 16], F32, tag="lnst", name="lnst")
            nc.vector.bn_stats(out=s[:, 0:6], in_=x_ap[:, 0:384])
            nc.vector.bn_stats(out=s[:, 6:12], in_=x_ap[:, 384:768])
            sv = s[:]
            nc.vector.bn_aggr(
                out=s[:, 12:14],
                in_=bass.AP(tensor=sv.tensor, offset=sv.offset,
                            ap=[sv.ap[0], [6, 2], [1, 6]]))
            nc.scalar.activation(out=s[:, 14:15], in_=s[:, 13:14],
                                 func=AF.Sqrt, bias=eps6[:])
            nc.vector.reciprocal(out=s[:, 14:15], in_=s[:, 14:15])
            nc.vector.tensor_scalar(
                out=out_ap, in0=x_ap, scalar1=s[:, 12:13], scalar2=s[:, 14:15],
                op0=OP.subtract, op1=OP.mult)

        def transpose4(srcs, dst_ap, on_act=False):
            """Transpose up to 4 [128,128] bf16 blocks via PE into one PSUM
            tile, then one copy into dst_ap ([128, 128*len] bf16) on DVE or
            (when DVE is the contended engine) the ACT engine."""
            ps = pp.tile([128, 512], BF16, tag="ps", name="pst")
            for k, s_ in enumerate(srcs):
                nc.tensor.transpose(ps[:, k * 128:(k + 1) * 128], s_, ident[:])
            if on_act:
                nc.scalar.activation(out=dst_ap, in_=ps[:, 0:128 * len(srcs)],
                                     func=AF.Identity)
            else:
                nc.vector.tensor_copy(out=dst_ap, in_=ps[:, 0:128 * len(srcs)])

        wqkvos = [None] * DEPTH

        def emit_wqkvo_dmas(j):
            lst = []
            for dc in range(DC):
                w_ = wp.tile([128, 3072], BF16, tag="w", name="w")
                nc.sync.dma_start(out=w_[:],
                                  in_=Lw[j]["wqkvo"][dc * 128:(dc + 1) * 128, :])
                lst.append(w_)
            wqkvos[j] = lst

        # ================= patch embed =================
        with nc.named_scope("embed"):
            # tiny latency-critical row: scalar HWDGE ring, ahead of the
            # weight-prefetch flood on the sync ring
            gr = ec.tile([1, 3 * D + 2 * G], F32, tag="grows", name="grows")
            nc.gpsimd.dma_start(out=gr[:], in_=grow[:1, :])
            cvb = sm.tile([1, D], BF16, tag="cvb", name="cvb", bufs=1)
            nc.gpsimd.dma_start(out=cvb[:], in_=convbr[:1, :])
            ps_e = {}
            for t in range(NT):
                for js in range(2):
                    ps_e[(t, js)] = pp.tile([128, 512], F32, tag="ps", name="ps")
            for dc in range(DC):
                xt = tr.tile([128, 256], BF16, tag="xt", name="xt", bufs=3)
                nc.sync.dma_start(out=xt[:, 0:N],
                                  in_=xcolT[dc * 128:(dc + 1) * 128, :])
                cwt = wp.tile([128, 3072], BF16, tag="w", name="w")
                nc.sync.dma_start(out=cwt[:, 0:384],
                                  in_=convw[dc * 128:(dc + 1) * 128, 0:384])
                nc.sync.dma_start(out=cwt[:, 384:768],
                                  in_=convw[dc * 128:(dc + 1) * 128, 384:768])
                for t in range(NT):
                    for js in range(2):
                        nc.tensor.matmul(
                            ps_e[(t, js)][:, 0:384],
                            xt[:, t * 128:(t + 1) * 128],
                            cwt[:, js * 384:(js + 1) * 384],
                            start=(dc == 0), stop=False)
            # layer-0 attention weights ahead of the late consts: the sync
            # HWDGE ring drains FIFO, and layer 0 needs wqkvo first
            emit_wqkvo_dmas(0)
            late_const_dmas()
            patches = [tr.tile([128, D], F32, tag="t", name="t") for _ in range(NT)]
            for t in range(NT):
                for js in range(2):
                    # + conv_b via K=1 ones-row matmul (exact)
                    nc.tensor.matmul(
                        ps_e[(t, js)][:, 0:384], ones_row[:1, :],
                        cvb[:1, js * 384:(js + 1) * 384],
                        start=False, stop=True)
                    nc.vector.tensor_copy(
                        out=patches[t][:, js * 384:(js + 1) * 384],
                        in_=ps_e[(t, js)][:, 0:384])

            # GroupNorm stats over (group channels x all tokens)
            part = [st.tile([128, 2 * G], F32, tag="gnp", name="gnp")
                    for _ in range(NT)]
            for t in range(NT):
                sq = tr.tile([128, D], F32, tag="t", name="t")
                nc.scalar.activation(out=sq[:], in_=patches[t][:], func=AF.Square)
                for g in range(G):
                    nc.vector.reduce_sum(out=part[t][:, g:g + 1],
                                         in_=patches[t][:, g * GS:(g + 1) * GS],
                                         axis=mybir.AxisListType.X)
                    nc.vector.reduce_sum(out=part[t][:, G + g:G + g + 1],
                                         in_=sq[:, g * GS:(g + 1) * GS],
                                         axis=mybir.AxisListType.X)
            partb = [st.tile([128, 2 * G], BF16, tag="gnpb", name="gnpb")
                     for _ in range(NT)]
            for t in range(NT):
                nc.vector.tensor_copy(out=partb[t][:], in_=part[t][:])
            psg = pp.tile([128, 512], F32, tag="ps", name="ps")
            for t in range(NT):
                nc.tensor.matmul(psg[0:1, 0:2 * G], ones_col[:], partb[t][:],
                                 start=(t == 0), stop=(t == NT - 1))
            # gr: [0:768] gn_g, [768:1536] gn_b, [1536:2304] scratch row,
            #     [2304:2320] group stats
            inv_cnt = 1.0 / (GS * N)
            nc.vector.tensor_scalar_mul(out=gr[:, 2304:2304 + 2 * G],
                                        in0=psg[0:1, 0:2 * G], scalar1=inv_cnt)
            mg = gr[:, 2304:2304 + G]
            msq = gr[:, 2304 + G:2304 + 2 * G]
            mg2 = gr[:, 1536:1536 + G]
            nc.vector.tensor_mul(out=mg2, in0=mg, in1=mg)
            nc.vector.tensor_sub(out=msq, in0=msq, in1=mg2)
            nc.scalar.activation(out=msq, in_=msq, func=AF.Sqrt,
                                 bias=eps5[0:1, :])
            nc.vector.reciprocal(out=msq, in_=msq)
            # A = rstd_g * gn_g ; Bc = gn_b - mean_g * A (per-group scalars,
            # expanded across each group's 96 channels via stride-0 APs)
            rsx = ec.tile([1, D], F32, tag="gscr", name="gscr")
            grv = gr[:]

            def _gexp(col):
                return bass.AP(tensor=grv.tensor, offset=grv.offset + col,
                               ap=[grv.ap[0], [1, G], [0, GS]])

            arow = gr[:, 1536:2304]
            nc.vector.tensor_tensor(out=arow, in0=gr[:, 0:D],
                                    in1=_gexp(2304 + G), op=OP.mult)
            nc.vector.tensor_tensor(out=rsx[:, 0:D], in0=arow,
                                    in1=_gexp(2304), op=OP.mult)
            nc.vector.tensor_sub(out=rsx[:, 0:D], in0=gr[:, D:2 * D],
                                 in1=rsx[:, 0:D])
            ab = lc.tile([128, 2 * D], F32, tag="gnab", name="gnab", bufs=1)
            nc.gpsimd.partition_broadcast(ab[:, 0:D], arow)
            nc.gpsimd.partition_broadcast(ab[:, D:2 * D], rsx[:1, 0:D])
            for t in range(NT):
                tmp = tr.tile([128, D], F32, tag="t", name="t")
                nc.vector.tensor_mul(out=tmp[:], in0=patches[t][:], in1=ab[:, 0:D])
                nc.vector.tensor_add(out=h[t][:], in0=tmp[:], in1=ab[:, D:2 * D])

        # ================= transformer layers =================
        # per-layer const rows: small DMAs on the scalar ring, prefetched one
        # layer ahead so the issue slot isn't stuck behind a whole layer of
        # ACT-queue work
        consts = [None] * DEPTH

        def emit_const_dmas(j):
            pj = Lw[j]
            lcb1 = lc.tile([128, 3 * D], BF16, tag="lcb1", name="lcb1")
            nc.gpsimd.dma_start(out=lcb1[:],
                                in_=_row_bcast(pj["lrow"][:1, 0:3 * D], 3 * D))
            lcb2 = lc.tile([128, D], BF16, tag="lcb2", name="lcb2")
            nc.gpsimd.dma_start(
                out=lcb2[:],
                in_=bass.AP(tensor=pj["lrow"][:1, :].tensor,
                            offset=pj["lrow"][:1, :].offset + 3 * D,
                            ap=[[0, 128], [1, D]]))
            borow = sm.tile([1, D], BF16, tag="borow", name="borow")
            nc.gpsimd.dma_start(out=borow[:], in_=pj["lrow"][:1, 4 * D:5 * D])
            smalls = sm.tile([128, 12 + MC], F32, tag="sme", name="sme")
            nc.gpsimd.dma_start(out=smalls[:], in_=pj["smalls"][:, :])
            consts[j] = (lcb1, lcb2, borow, smalls)

        emit_const_dmas(0)
        pend = None
        for i in range(DEPTH):
            p = Lw[i]
            with nc.named_scope(f"layer{i}"):
                lcb1, lcb2, borow, smalls = consts[i]
                SHIFT = lcb1[:, 0:D]
                MOD1 = lcb1[:, D:2 * D]
                BV = lcb1[:, 2 * D:3 * D]
                B2 = lcb2[:, 0:D]
                BQP = smalls[:, 0:6]     # permuted q bias, col dc = chunk
                BKP = smalls[:, 6:12]
                B1C = smalls[:, 12:12 + MC]

                # weight tiles (prefetchable large DMAs, sync ring)
                if wqkvos[i] is None:
                    emit_wqkvo_dmas(i)
                wqkvo = wqkvos[i]

                # --- finalize h from previous layer's MLP + AdaLN + LN1 ---
                def finalize_h(t):
                    if pend is not None:
                        pps2, ph1B2 = pend
                        for js in range(2):
                            sl = slice(js * 384, (js + 1) * 384)
                            nc.vector.tensor_add(out=h[t][:, sl],
                                                 in0=pps2[(t, js)][:, 0:384],
                                                 in1=ph1B2[t][:, sl])

                hmod = [res.tile([128, D], F32, tag="res", name="res")
                        for _ in range(NT)]
                hn = [tr.tile([128, D], BF16, tag="hn", name="hn", bufs=4)
                      for _ in range(NT)]

                def ada_chain(t):
                    tmp = tr.tile([128, D], F32, tag="t", name="t")
                    ln_apply(h[t][:], tmp[:])
                    tmp2 = tr.tile([128, D], F32, tag="t", name="t")
                    nc.vector.tensor_mul(out=tmp2[:], in0=tmp[:], in1=MOD1)
                    nc.vector.tensor_add(out=hmod[t][:], in0=tmp2[:], in1=SHIFT)
                    ln_apply(hmod[t][:], hn[t][:])

                hnT = wt.tile([128, DC * N], BF16, tag="wt", name="wt")

                def hnT_transposes(t, dst, src):
                    transpose4([src[t][:, dc * 128:(dc + 1) * 128]
                                for dc in range(4)],
                               _ap3(dst[:], t * 128, 4, 256, 128), on_act=True)
                    transpose4([src[t][:, dc * 128:(dc + 1) * 128]
                                for dc in range(4, 6)],
                               _ap3(dst[:], 4 * 256 + t * 128, 2, 256, 128),
                               on_act=True)

                ps_v = {}

                def v_gemm(t):
                    for js in range(2):
                        ps_v[(t, js)] = pp.tile([128, 512], F32, tag="ps",
                                                name="ps")
                    for dc in range(DC):
                        for js in range(2):
                            nc.tensor.matmul(
                                ps_v[(t, js)][:, 0:384],
                                hnT[:, dc * N + t * 128:dc * N + (t + 1) * 128],
                                wqkvo[dc][:, 2 * D + js * 384:2 * D + (js + 1) * 384],
                                start=(dc == 0), stop=(dc == DC - 1))

                def v_aug_add(t):
                    for js in range(2):
                        nc.vector.tensor_tensor(
                            out=_ap3(v_aug[t][:], js * 6 * 66, 6, 66, 64),
                            in0=_ap3(ps_v[(t, js)][:, 0:384], 0, 6, 64, 64),
                            in1=_ap3(BV, js * 384, 6, 64, 64), op=OP.add)

                # packed Q|K PSUM per qc: cols [0:256]=Q, [256:512]=K,
                # each accumulated per token tile t (so tile t0's GEMMs can
                # run while tile t1's AdaLN chain is still on the DVE).
                psqk = [None] * DC

                def qk_gemm(t, qcs):
                    for qc in qcs:
                        if psqk[qc] is None:
                            psqk[qc] = pp.tile([128, 512], F32, tag="ps",
                                               name="ps")
                        for coff, kbase in ((0, 0), (D, 256)):
                            for dc in range(DC):
                                nc.tensor.matmul(
                                    psqk[qc][:, kbase + t * 128:
                                             kbase + (t + 1) * 128],
                                    wqkvo[dc][:, coff + qc * 128:
                                              coff + (qc + 1) * 128],
                                    hnT[:, dc * N + t * 128:
                                        dc * N + (t + 1) * 128],
                                    start=(dc == 0), stop=(dc == DC - 1))

                rotqk = [None] * DC

                def rot_chunk(qc):
                    """q|k rotary for one 128-chunk, 512-wide merged ops."""
                    qb = rt.tile([128, 512], BF16, tag="qb", name="qb")
                    nc.scalar.activation(out=qb[:, 0:256],
                                         in_=psqk[qc][:, 0:256],
                                         func=AF.Identity,
                                         bias=BQP[:, qc:qc + 1])
                    nc.scalar.activation(out=qb[:, 256:512],
                                         in_=psqk[qc][:, 256:512],
                                         func=AF.Identity,
                                         bias=BKP[:, qc:qc + 1])
                    sh = rt.tile([128, 512], BF16, tag="qb", name="qb")
                    nc.vector.stream_shuffle(sh[:], qb[:], SWAP_MASK)
                    u = rt.tile([128, 512], BF16, tag="qb", name="qb")
                    nc.vector.tensor_mul(out=u[:], in0=qb[:], in1=cosPt[:])
                    nc.vector.tensor_mul(out=sh[:], in0=sh[:], in1=sinPt[:])
                    rotqk[qc] = rq.tile([128, 512], BF16, tag="rq", name="rq")
                    nc.vector.tensor_add(out=rotqk[qc][:], in0=u[:], in1=sh[:])

                # ---- boundary + QKV, t-pipelined ----
                finalize_h(0)
                ada_chain(0)
                hnT_transposes(0, hnT, hn)
                v_gemm(0)
                qk_gemm(0, range(0, 4))
                finalize_h(1)
                ada_chain(1)
                # anchored dummy exp: pulls the exp-set ACT table load to
                # right after ada t1 (hidden under the QK GEMMs); the anchor
                # input stops the scheduler from hoisting it
                nc.scalar.activation(out=dmy[:1, 0:1], in_=hn[1][0:1, 0:1],
                                     func=AF.Exp)
                qk_gemm(0, range(4, DC))
                v_aug_add(0)
                hnT_transposes(1, hnT, hn)
                qk_gemm(1, [0])
                rot_chunk(0)
                v_gemm(1)
                v_aug_add(1)
                for qc in range(1, DC):
                    qk_gemm(1, [qc])
                    rot_chunk(qc)

                # ---- attention ----
                attn = [tr.tile([128, D], BF16, tag="at", name="at", bufs=2)
                        for _ in range(NT)]
                attnT = wt.tile([128, DC * N], BF16, tag="wt", name="wt")
                ps_o = {}

                def scores(hd_):
                    jc = hd_ // 2
                    po = (hd_ % 2) * 64
                    ps = pp.tile([128, 512], F32, tag="ps", name="ps")
                    for mc in range(NT):
                        nc.tensor.matmul(
                            ps[:, mc * 256:(mc + 1) * 256],
                            rotqk[jc][po:po + 64,
                                      256 + mc * 128:256 + (mc + 1) * 128],
                            rotqk[jc][po:po + 64, 0:256],
                            start=True, stop=True)
                    return ps

                pa = {}
                ps_s = scores(0)
                for hd_ in range(NH):
                    half = hd_ // 6
                    hi = hd_ % 6
                    if hi == 0:
                        for t in range(NT):
                            pa[(half, t)] = pp.tile([128, 512], F32, tag="ps",
                                                    name="ps")
                    es = ex.tile([128, 512], BF16, tag="ex", name="ex")
                    nc.scalar.activation(out=es[:], in_=ps_s[:, 0:512],
                                         func=AF.Exp, scale=HD ** -0.5)
                    if hd_ + 1 < NH:
                        ps_s = scores(hd_ + 1)
                    for t in range(NT):
                        for mc in range(NT):
                            nc.tensor.matmul(
                                pa[(half, t)][:, hi * 66:(hi + 1) * 66],
                                es[:, mc * 256 + t * 128:mc * 256 + (t + 1) * 128],
                                v_aug[mc][:, hd_ * 66:(hd_ + 1) * 66],
                                start=(mc == 0), stop=(mc == NT - 1))
                    if hi == 5:
                        # batched softmax normalize for this 6-head group
                        for t in range(NT):
                            pav = pa[(half, t)][:]
                            rz = st.tile([128, 8], F32, tag="rz", name="rz")
                            nc.vector.reciprocal(
                                out=rz[:, 0:6],
                                in_=bass.AP(tensor=pav.tensor,
                                            offset=pav.offset + 64,
                                            ap=[pav.ap[0], [66, 6], [1, 1]]))
                            rzv = rz[:]
                            nc.vector.tensor_tensor(
                                out=_ap3(attn[t][:], half * 384, 6, 64, 64),
                                in0=_ap3(pav, 0, 6, 66, 64),
                                in1=bass.AP(tensor=rzv.tensor,
                                            offset=rzv.offset,
                                            ap=[rzv.ap[0], [1, 6], [0, 64]]),
                                op=OP.mult)
                        for jc in range(3 * half, 3 * half + 3):
                            transpose4(
                                [attn[t][:, jc * 128:(jc + 1) * 128]
                                 for t in range(NT)],
                                attnT[:, jc * N:(jc + 1) * N])
                        if half == 0:
                            # t0-only out-proj for the first half's chunks
                            for js in range(2):
                                ps_o[(0, js)] = pp.tile([128, 512], F32,
                                                        tag="ps", name="ps")
                            for dc in range(0, 3):
                                for js in range(2):
                                    nc.tensor.matmul(
                                        ps_o[(0, js)][:, 0:384],
                                        attnT[:, dc * N:dc * N + 128],
                                        wqkvo[dc][:, 3 * D + js * 384:
                                                  3 * D + (js + 1) * 384],
                                        start=(dc == 0), stop=False)
                        else:
                            # finish t0 (incl. folded bo bias) so its LN2
                            # chain runs while the PE does ALL of t1's
                            # out-proj chunks
                            for dc in range(3, DC):
                                for js in range(2):
                                    nc.tensor.matmul(
                                        ps_o[(0, js)][:, 0:384],
                                        attnT[:, dc * N:dc * N + 128],
                                        wqkvo[dc][:, 3 * D + js * 384:
                                                  3 * D + (js + 1) * 384],
                                        start=False, stop=False)
                            for js in range(2):
                                nc.tensor.matmul(
                                    ps_o[(0, js)][:, 0:384],
                                    ones_row[:1, :],
                                    borow[:1, js * 384:(js + 1) * 384],
                                    start=False, stop=True)
                            for js in range(2):
                                ps_o[(1, js)] = pp.tile([128, 512], F32,
                                                        tag="ps", name="ps")
                            for dc in range(DC):
                                for js in range(2):
                                    nc.tensor.matmul(
                                        ps_o[(1, js)][:, 0:384],
                                        attnT[:, dc * N + 128:
                                              dc * N + 256],
                                        wqkvo[dc][:, 3 * D + js * 384:
                                                  3 * D + (js + 1) * 384],
                                        start=(dc == 0), stop=False)
                            for js in range(2):
                                nc.tensor.matmul(
                                    ps_o[(1, js)][:, 0:384],
                                    ones_row[:1, :],
                                    borow[:1, js * 384:(js + 1) * 384],
                                    start=False, stop=True)

                # ---- MLP, t-pipelined ----
                w1l = []
                for dc in range(DC):
                    w_ = wp.tile([128, 3072], BF16, tag="w", name="w")
                    nc.sync.dma_start(out=w_[:],
                                      in_=p["w1"][dc * 128:(dc + 1) * 128, :])
                    w1l.append(w_)
                w2l = []
                for k in range(DC):
                    w_ = wp.tile([128, 3072], BF16, tag="w", name="w")
                    nc.sync.dma_start(out=w_[:],
                                      in_=p["w2p"][:, k * 3072:(k + 1) * 3072])
                    w2l.append(w_)

                h1 = [res.tile([128, D], F32, tag="res", name="res")
                      for _ in range(NT)]
                h1B2 = [res.tile([128, D], F32, tag="res", name="res")
                        for _ in range(NT)]
                hn2 = [tr.tile([128, D], BF16, tag="hn", name="hn", bufs=4)
                       for _ in range(NT)]
                hn2T = wt.tile([128, DC * N], BF16, tag="wt", name="wt")

                def ln2_chain(t):
                    for js in range(2):
                        sl = slice(js * 384, (js + 1) * 384)
                        nc.vector.tensor_add(out=h1[t][:, sl],
                                             in0=ps_o[(t, js)][:, 0:384],
                                             in1=hmod[t][:, sl])
                    ln_apply(h1[t][:], hn2[t][:])

                def hn2T_transposes(t):
                    transpose4([hn2[t][:, dc * 128:(dc + 1) * 128]
                                for dc in range(4)],
                               _ap3(hn2T[:], t * 128, 4, 256, 128))
                    transpose4([hn2[t][:, dc * 128:(dc + 1) * 128]
                                for dc in range(4, 6)],
                               _ap3(hn2T[:], 4 * 256 + t * 128, 2, 256, 128))

                ps1p = {}
                g_l = [None] * MC

                def mlp_up(mc, t):
                    k = mc // 2
                    if mc % 2 == 0 and t == 0:
                        ps1p[k] = pp.tile([128, 512], F32, tag="ps", name="ps")
                    base = (mc % 2) * 256 + t * 128
                    for dc in range(DC):
                        nc.tensor.matmul(
                            ps1p[k][:, base:base + 128],
                            w1l[dc][:, mc * 128:(mc + 1) * 128],
                            hn2T[:, dc * N + t * 128:dc * N + (t + 1) * 128],
                            start=(dc == 0), stop=(dc == DC - 1))

                def mlp_gelu(mc):
                    g_ = ge.tile([128, 256], BF16, tag="ge", name="ge")
                    nc.scalar.activation(
                        out=g_[:], in_=ps1p[mc // 2][:, (mc % 2) * 256:
                                                     (mc % 2) * 256 + 256],
                        func=AF.Gelu, bias=B1C[:, mc:mc + 1])
                    g_l[mc] = g_

                ps2 = {}

                def mlp_down(mc, t):
                    for js in range(2):
                        if mc == 0:
                            ps2[(t, js)] = pp.tile([128, 512], F32, tag="ps",
                                                   name="ps")
                        nc.tensor.matmul(
                            ps2[(t, js)][:, 0:384],
                            g_l[mc][:, t * 128:(t + 1) * 128],
                            w2l[mc // 4][:, (mc % 4) * D + js * 384:
                                         (mc % 4) * D + (js + 1) * 384],
                            start=(mc == 0), stop=(mc == MC - 1))

                ln2_chain(0)
                hn2T_transposes(0)
                for mc in range(U0):
                    mlp_up(mc, 0)
                ln2_chain(1)
                # anchored dummy: pull the gelu ACT table load to right after
                # ln2 t1, hidden under the up GEMMs
                nc.scalar.activation(out=dmy[:1, 0:1], in_=hn2[1][0:1, 0:1],
                                     func=AF.Gelu)
                hn2T_transposes(1)
                if i + 1 < DEPTH:
                    emit_const_dmas(i + 1)
                for mc in range(MC):
                    mlp_up(mc, 1)
                    mlp_gelu(mc)
                    if mc + U0 < MC:
                        mlp_up(mc + U0, 0)
                    if mc >= 1:
                        mlp_down(mc - 1, 0)
                mlp_down(MC - 1, 0)
                # h1B2 late so the scheduler cannot slot these ahead of the
                # critical ln2 tensor_scalar ops on the DVE
                for t in range(NT):
                    nc.vector.tensor_add(out=h1B2[t][:], in0=h1[t][:], in1=B2)
                for mc in range(MC):
                    mlp_down(mc, 1)
                pend = (ps2, h1B2)

        # ================= final layer =================
        with nc.named_scope("final"):
            ob = res.tile([128, D], F32, tag="res", name="ob")
            nc.gpsimd.dma_start(out=ob[:], in_=_row_bcast(outrow[:1, :], D))
            hf = [tr.tile([128, D], BF16, tag="hn", name="hn", bufs=4)
                  for _ in range(NT)]
            fps2, fh1B2 = pend
            owl = []
            for dc in range(DC):
                w_ = wp.tile([128, 3072], BF16, tag="w", name="w")
                nc.sync.dma_start(out=w_[:, 0:D],
                                  in_=outw[dc * 128:(dc + 1) * 128, :])
                owl.append(w_)
            hfT = wt.tile([128, DC * N], BF16, tag="wt", name="wt")
            ps_f = {}

            def fin_chain(t):
                for js in range(2):
                    sl = slice(js * 384, (js + 1) * 384)
                    nc.vector.tensor_add(out=h[t][:, sl],
                                         in0=fps2[(t, js)][:, 0:384],
                                         in1=fh1B2[t][:, sl])
                ln_apply(h[t][:], hf[t][:])

            def fin_transposes(t):
                transpose4([hf[t][:, dc * 128:(dc + 1) * 128]
                            for dc in range(4)],
                           _ap3(hfT[:], t * 128, 4, 256, 128), on_act=True)
                transpose4([hf[t][:, dc * 128:(dc + 1) * 128]
                            for dc in range(4, 6)],
                           _ap3(hfT[:], 4 * 256 + t * 128, 2, 256, 128),
                           on_act=True)

            def fin_gemm(t):
                for js in range(2):
                    ps_f[(t, js)] = pp.tile([128, 512], F32, tag="ps", name="ps")
                for dc in range(DC):
                    for js in range(2):
                        nc.tensor.matmul(
                            ps_f[(t, js)][:, 0:384],
                            hfT[:, dc * N + t * 128:dc * N + (t + 1) * 128],
                            owl[dc][:, js * 384:(js + 1) * 384],
                            start=(dc == 0), stop=(dc == DC - 1))

            def fin_out(t):
                osb = tr.tile([128, D], F32, tag="t", name="t")
                for js in range(2):
                    sl = slice(js * 384, (js + 1) * 384)
                    nc.vector.tensor_add(out=osb[:, sl],
                                         in0=ps_f[(t, js)][:, 0:384],
                                         in1=ob[:, sl])
                nc.sync.dma_start(out=out[t * 128:(t + 1) * 128, :], in_=osb[:])

            fin_chain(0)
            fin_transposes(0)
            fin_gemm(0)
            fin_chain(1)
            fin_transposes(1)
            fin_gemm(1)
            fin_out(0)
            fin_out(1)


# ---------------------------------------------------------------- host side

def _host_prep(inputs):
    f32 = np.float32
    x = np.asarray(inputs["x"], f32)
    t = np.asarray(inputs["t"], f32)

    # time embedding + AdaLN modulation (sidecar, ~0.25% of model FLOPs)
    ts = t * 1000.0
    half = 384
    freqs = np.exp(np.arange(half, dtype=f32) * f32(-math.log(10000.0) / (half - 1)))
    e = ts[:, None] * freqs[None, :]
    temb = np.concatenate([np.sin(e), np.cos(e)], axis=-1).astype(f32)
    a = temb @ np.asarray(inputs["t_w1"], f32) + np.asarray(inputs["t_b1"], f32)
    a = (a / (1.0 + np.exp(-a))).astype(f32)  # silu
    temb = (a @ np.asarray(inputs["t_w2"], f32)
            + np.asarray(inputs["t_b2"], f32)).astype(f32)
    stemb = (temb / (1.0 + np.exp(-temb))).astype(f32)  # silu(temb)
    ada_w = np.asarray(inputs["ada_w"], f32)
    ada_b = np.asarray(inputs["ada_b"], f32)
    sc = np.einsum("bk,iko->bio", stemb, ada_w).astype(f32) + ada_b[None]
    shift = sc[:, :, :D]
    mod1 = (1.0 + sc[:, :, D:]).astype(f32)

    # im2col (transposed): xcolT[b] [(c p q), n]
    xr = x.reshape(B, C_IN, HH // P, P, WW // P, P)
    xcol = xr.transpose(0, 2, 4, 1, 3, 5).reshape(B, N, D)
    xcolT = np.ascontiguousarray(xcol.transpose(0, 2, 1))

    convw = np.ascontiguousarray(np.asarray(inputs["conv_w"], f32).reshape(D, D).T)
    convbr = np.asarray(inputs["conv_b"], f32)[None]

    grow = np.zeros((1, 3 * D + 2 * G), f32)
    grow[0, 0:D] = np.asarray(inputs["gn_g"], f32)
    grow[0, D:2 * D] = np.asarray(inputs["gn_b"], f32)

    # rotary pair-interleaved permutation: within each head's 64 dims,
    # output order is [0, 32, 1, 33, ..., 31, 63]
    perm64 = np.empty(64, np.int64)
    perm64[0::2] = np.arange(32)
    perm64[1::2] = np.arange(32, 64)
    permD = np.concatenate([hh * 64 + perm64 for hh in range(NH)])

    # rotary tables in permuted transposed layout [128, N] (head pair)
    inv = (10000.0 ** (-(np.arange(0, HD, 2, dtype=f32)) / HD)).astype(f32)
    f_ = np.arange(N, dtype=f32)[:, None] * inv[None, :]  # [N, 32]
    cos_t = np.cos(f_).astype(f32)   # [N, 32]
    sin_t = np.sin(f_).astype(f32)
    cosP = np.empty((128, N), f32)
    sinP = np.empty((128, N), f32)
    for pidx in range(64):
        i_ = pidx // 2
        cosP[pidx] = cos_t[:, i_]
        sinP[pidx] = sin_t[:, i_] * (-1.0 if pidx % 2 == 0 else 1.0)
    cosP[64:] = cosP[:64]
    sinP[64:] = sinP[:64]
    cosPP = np.concatenate([cosP, cosP], axis=1)  # [128, 2N]: q|k merged
    sinPP = np.concatenate([sinP, sinP], axis=1)

    ln1_g = np.asarray(inputs["ln1_g"], f32)
    ln1_b = np.asarray(inputs["ln1_b"], f32)
    ln2_g = np.asarray(inputs["ln2_g"], f32)
    ln2_b = np.asarray(inputs["ln2_b"], f32)

    layers = []
    for i in range(DEPTH):
        wq = np.asarray(inputs["wq"][i], f32)
        wk = np.asarray(inputs["wk"][i], f32)
        wv = np.asarray(inputs["wv"][i], f32)
        wo = np.asarray(inputs["wo"][i], f32)
        g1 = ln1_g[i][:, None]
        bq = np.asarray(inputs["bq"][i], f32) + ln1_b[i] @ wq
        bk = np.asarray(inputs["bk"][i], f32) + ln1_b[i] @ wk
        bv = np.asarray(inputs["bv"][i], f32) + ln1_b[i] @ wv
        # permute q/k output columns for pair-interleaved rotary
        wqp = (g1 * wq)[:, permD]
        wkp = (g1 * wk)[:, permD]
        bqp = bq[permD]
        bkp = bk[permD]
        wqkvo = np.concatenate([wqp, wkp, g1 * wv, wo], axis=1).astype(f32)
        w1 = np.asarray(inputs["w1"][i], f32)
        w2 = np.asarray(inputs["w2"][i], f32)
        # w2 pre-chunked: [128, 24*768], block mc = w2[mc*128:(mc+1)*128, :]
        w2p = np.ascontiguousarray(
            w2.reshape(MC, 128, D).transpose(1, 0, 2).reshape(128, MC * D))
        b1 = (np.asarray(inputs["b1"][i], f32) + ln2_b[i] @ w1).astype(f32)
        smalls = np.zeros((128, 12 + MC), f32)
        smalls[:, 0:6] = bqp.reshape(6, 128).T
        smalls[:, 6:12] = bkp.reshape(6, 128).T
        smalls[:, 12:12 + MC] = b1.reshape(MC, 128).T
        bo = np.asarray(inputs["bo"][i], f32)
        b2 = np.asarray(inputs["b2"][i], f32)
        lrow = np.concatenate([
            np.zeros(D, f32), np.zeros(D, f32),  # shift, mod1 filled per-batch
            bv, b2, bo]).astype(f32)[None]
        layers.append(dict(
            wqkvo=np.ascontiguousarray(wqkvo),
            w1=np.ascontiguousarray((ln2_g[i][:, None] * w1).astype(f32)),
            w2p=w2p,
            lrow=lrow,
            smalls=smalls,
        ))

    out_w = np.asarray(inputs["out_w"], f32)
    outw = np.ascontiguousarray(
        (np.asarray(inputs["fin_g"], f32)[:, None] * out_w).astype(f32))
    outrow = (np.asarray(inputs["out_b"], f32)
              + np.asarray(inputs["fin_b"], f32) @ out_w).astype(f32)[None]

    import ml_dtypes
    bfc = lambda a: np.ascontiguousarray(a.astype(ml_dtypes.bfloat16))
    in_maps = []
    for b in range(B):
        m = dict(
            xcolT=bfc(xcolT[b]),
            identm=bfc(np.eye(128, dtype=f32)),
            onesr=bfc(np.ones((1, 128), f32)),
            convw=bfc(convw), convbr=bfc(convbr), grow=grow,
            cosPP=bfc(cosPP), sinPP=bfc(sinPP), outw=bfc(outw), outrow=outrow,
        )
        for i, L in enumerate(layers):
            m[f"wqkvo{i}"] = bfc(L["wqkvo"])
            m[f"w1{i}"] = bfc(L["w1"])
            m[f"w2p{i}"] = bfc(L["w2p"])
            lr = L["lrow"].copy()
            lr[0, 0:D] = shift[b, i]
            lr[0, D:2 * D] = mod1[b, i]
            m[f"lrow{i}"] = bfc(lr)
            m[f"smalls{i}"] = L["smalls"]
        in_maps.append(m)
    return in_maps


def kernel(**inputs):
    if "nc" not in _CACHE:
        _CACHE["nc"] = _build()
    nc = _CACHE["nc"]
    in_maps = _host_prep(inputs)
    trace = bool(os.environ.get("KERNEL_TRACE"))
    res = run_bass_kernel_spmd(nc, in_maps, list(range(B)), trace=trace)
    LAST_RESULT["res"] = res
    out = np.empty((B, C_IN, HH, WW), np.float32)
    for b in range(B):
        o = res.results[b]["out"]  # [256, 768] = [n, (c p q)]
        out[b] = (o.reshape(16, 16, C_IN, P, P)
                  .transpose(2, 0, 3, 1, 4).reshape(C_IN, HH, WW))
    return out


if __name__ == "__main__":
    _build()
    print("build ok")


# revision 25
# speedup vs baseline: 1.2117x; 1.2117x over previous
"""Trainium2 Bass kernel for nn_DiT_4758823763997 (DiT dense transformer).

B=8 batch, N=256 tokens, D=768, 12 layers, 12 heads (hd 64), MLP 3072.
Sharding: pure data-parallel - one batch element per NeuronCore (8 cores),
weights replicated; no collectives.

v3 design (vs v2):
  - Per-token-tile (t0/t1) software pipelining across ALL phase boundaries:
    the serial DVE LayerNorm/AdaLN chains for tile t run while the PE works
    on the other tile's GEMMs (prev-layer mlp-down t1, per-t QK, per-t
    mlp-up), eliminating the two ~8us PE stalls per layer.
  - LN rstd computed as exp(-0.5*ln(var+eps)): Ln and Exp share one ACT
    table set, so the only ACT table swaps left are gelu<->ln/exp (2 per
    layer), both prefetched via dummy ops while ACT is idle.
  - Rotary processed 512-wide (q|k merged per chunk) on DVE in bf16.
  - Softmax normalizer batched per 6-head group: AV results accumulate in
    one PSUM bank per (t, half), one strided reciprocal + one stride-0
    broadcast multiply replace 24 reciprocal+scale pairs.
  - Out-proj bias folded into the GEMM accumulation (ones-row matmul).
  - Small latency-critical DMAs (per-layer rows, GN row) issued on the
    scalar HWDGE ring so the weight-prefetch flood on the sync ring cannot
    delay them.
"""

import math
import os
import sys

sys.path.insert(0, "/opt/trn_rl_repo")

import numpy as np

import concourse.bass as bass
import concourse.bacc as bacc
import concourse.mybir as mybir
import concourse.tile as tile
from concourse.bass_utils import run_bass_kernel_spmd

B = 8
C_IN = 3
HH = 256
WW = 256
P = 16
D = 768
DEPTH = 12
NH = 12
HD = 64
MLPD = 3072
N = 256
G = 8
GS = D // G

F32 = mybir.dt.float32
BF16 = mybir.dt.bfloat16
AF = mybir.ActivationFunctionType
OP = mybir.AluOpType

DC = D // 128    # 6
NT = N // 128    # 2
MC = MLPD // 128  # 24
U0 = 6           # mlp-up t0 runahead chunks (even; bounds live PSUM pairs)

LAST_RESULT = {}
_CACHE = {}

# stream_shuffle mask: swap adjacent partitions within each 32-quadrant
SWAP_MASK = [i ^ 1 for i in range(32)]


def _ap3(ap2d, base, nblk, stride, width):
    """[128, nblk, width] free-strided view of a 2D AP at column offset base."""
    return bass.AP(tensor=ap2d.tensor, offset=ap2d.offset + base,
                   ap=[ap2d.ap[0], [stride, nblk], [1, width]])


def _row_bcast(row_ap, width, parts=128):
    """[1, W] row -> step-0 partition-broadcast AP [parts, W]."""
    return bass.AP(tensor=row_ap.tensor, offset=row_ap.offset,
                   ap=[[0, parts], [1, width]])


def _build():
    nc = bacc.Bacc("TRN2", target_bir_lowering=False, debug=False, num_devices=8)

    def din(name, shape, dt=BF16):
        return nc.declare_dram_parameter(name, list(shape), dt, isOutput=False)

    xcolT = din("xcolT", [D, N])
    identm = din("identm", [128, 128])
    onesr = din("onesr", [1, 128])
    convw = din("convw", [D, D])
    convbr = din("convbr", [1, D])
    grow = din("grow", [1, 3 * D + 2 * G], F32)   # gn_g | gn_b | scratch
    cosPP = din("cosPP", [128, 2 * N])
    sinPP = din("sinPP", [128, 2 * N])
    Lw = []
    for i in range(DEPTH):
        Lw.append(dict(
            wqkvo=din(f"wqkvo{i}", [D, 4 * D]),        # wq|wk|wv|wo (q,k col-permuted)
            w1=din(f"w1{i}", [D, MLPD]),
            w2p=din(f"w2p{i}", [128, MC * D]),          # pre-chunked [128, 24*768]
            lrow=din(f"lrow{i}", [1, 4 * D]),           # shift+bo|mod1|bv|b2 (bf16)
            smalls=din(f"smalls{i}", [128, 12 + MC], F32),  # bqP|bkP|b1c
        ))
    outw = din("outw", [D, D])
    outrow = din("outrow", [1, D], F32)
    out = nc.declare_dram_parameter("out", [N, D], F32, isOutput=True)

    with tile.TileContext(nc) as tc:
        _emit(nc, tc, xcolT, identm, onesr, convw, convbr, grow, cosPP, sinPP,
              Lw, outw, outrow, out)
    nc.compile()
    return nc


def _emit(nc, tc, xcolT, identm, onesr, convw, convbr, grow, cosPP, sinPP,
          Lw, outw, outrow, out):
    from contextlib import ExitStack
    with ExitStack() as ctx:
        pers = ctx.enter_context(tc.tile_pool(name="pers", bufs=1))
        wp = ctx.enter_context(tc.tile_pool(name="wp", bufs=13))     # [128,3072] bf16 weight tiles
        res = ctx.enter_context(tc.tile_pool(name="res", bufs=6))
        tr = ctx.enter_context(tc.tile_pool(name="tr", bufs=4))
        wt = ctx.enter_context(tc.tile_pool(name="wt", bufs=4))      # transposed activations bf16
        rq = ctx.enter_context(tc.tile_pool(name="rq", bufs=7))      # rotated q|k chunks
        rt = ctx.enter_context(tc.tile_pool(name="rt", bufs=4))      # rotary transients
        st = ctx.enter_context(tc.tile_pool(name="st", bufs=6))
        ex = ctx.enter_context(tc.tile_pool(name="ex", bufs=3))
        ge = ctx.enter_context(tc.tile_pool(name="ge", bufs=26))     # gelu chunks (24 live)
        lc = ctx.enter_context(tc.tile_pool(name="lc", bufs=2))      # bcast rows bf16
        sm = ctx.enter_context(tc.tile_pool(name="sm", bufs=2))      # smalls
        ec = ctx.enter_context(tc.tile_pool(name="ec", bufs=1))
        pp = ctx.enter_context(tc.tile_pool(name="pp", bufs=8, space="PSUM"))

        ident = pers.tile([128, 128], BF16, tag="ident", name="ident")
        ones_col = pers.tile([128, 1], BF16, tag="onesc", name="onesc")
        nc.sync.dma_start(
            out=ones_col[:],
            in_=bass.AP(tensor=onesr[:1, :].tensor, offset=onesr[:1, :].offset,
                        ap=[[0, 128], [1, 1]]))
        ones_row = pers.tile([1, 128], BF16, tag="onesr", name="onesr")
        nc.sync.dma_start(out=ones_row[:], in_=onesr[:1, :])
        eps6 = pers.tile([128, 1], F32, tag="eps6", name="eps6")
        nc.vector.memset(eps6[:], 1e-6)
        eps5 = pers.tile([128, 1], F32, tag="eps5", name="eps5")
        nc.vector.memset(eps5[:], 1e-5)
        dmy = pers.tile([1, 2], F32, tag="dmy", name="dmy")
        nc.vector.memset(dmy[:], 1.0)

        cosPt = pers.tile([128, 2 * N], BF16, tag="cosPP", name="cosPP")
        sinPt = pers.tile([128, 2 * N], BF16, tag="sinPP", name="sinPP")

        h = [pers.tile([128, D], F32, tag=f"h{t}", name=f"h{t}") for t in range(NT)]
        v_aug = [pers.tile([128, NH * 66], BF16, tag=f"va{t}", name=f"va{t}")
                 for t in range(NT)]

        def late_const_dmas():
            # constants not needed until layer 0: emitted after the embed's
            # input DMAs so they don't delay the first conv matmuls
            nc.sync.dma_start(out=ident[:], in_=identm[:, :])
            nc.sync.dma_start(out=cosPt[:], in_=cosPP[:, :])
            nc.sync.dma_start(out=sinPt[:], in_=sinPP[:, :])
            for t in range(NT):
                va = v_aug[t][:]
                nc.sync.dma_start(
                    out=bass.AP(tensor=va.tensor, offset=va.offset + 64,
                                ap=[va.ap[0], [66, NH], [1, 2]]),
                    in_=bass.AP(tensor=onesr[:1, :].tensor,
                                offset=onesr[:1, :].offset,
                                ap=[[0, 128], [1, 2 * NH]]))

        def ln_apply(x_ap, out_ap):
            """out = (x - mean)/sqrt(var + 1e-6) along free dim 768."""
            s = st.tile([128, 16], F32, tag="lnst", name="lnst")
            nc.vector.bn_stats(out=s[:, 0:6], in_=x_ap[:, 0:384])
            nc.vector.bn_stats(out=s[:, 6:12], in_=x_ap[:, 384:768])
            sv = s[:]
            nc.vector.bn_aggr(
                out=s[:, 12:14],
                in_=bass.AP(tensor=sv.tensor, offset=sv.offset,
                            ap=[sv.ap[0], [6, 2], [1, 6]]))
            nc.scalar.activation(out=s[:, 14:15], in_=s[:, 13:14],
                                 func=AF.Sqrt, bias=eps6[:])
            nc.vector.reciprocal(out=s[:, 14:15], in_=s[:, 14:15])
            nc.vector.tensor_scalar(
                out=out_ap, in0=x_ap, scalar1=s[:, 12:13], scalar2=s[:, 14:15],
                op0=OP.subtract, op1=OP.mult)

        def transpose4(srcs, dst_ap, on_act=False):
            """Transpose up to 4 [128,128] bf16 blocks via PE into one PSUM
            tile, then one copy into dst_ap ([128, 128*len] bf16) on DVE or
            (when DVE is the contended engine) the ACT engine."""
            ps = pp.tile([128, 512], BF16, tag="ps", name="pst")
            for k, s_ in enumerate(srcs):
                nc.tensor.transpose(ps[:, k * 128:(k + 1) * 128], s_, ident[:])
            if on_act:
                nc.scalar.activation(out=dst_ap, in_=ps[:, 0:128 * len(srcs)],
                                     func=AF.Identity)
            else:
                nc.vector.tensor_copy(out=dst_ap, in_=ps[:, 0:128 * len(srcs)])

        wqkvos = [None] * DEPTH

        def emit_wqkvo_dmas(j):
            lst = []
            for dc in range(DC):
                w_ = wp.tile([128, 3072], BF16, tag="w", name="w")
                nc.sync.dma_start(out=w_[:],
                                  in_=Lw[j]["wqkvo"][dc * 128:(dc + 1) * 128, :])
                lst.append(w_)
            wqkvos[j] = lst

        # ================= patch embed =================
        with nc.named_scope("embed"):
            # tiny latency-critical row: scalar HWDGE ring, ahead of the
            # weight-prefetch flood on the sync ring
            gr = ec.tile([1, 3 * D + 2 * G], F32, tag="grows", name="grows")
            nc.gpsimd.dma_start(out=gr[:], in_=grow[:1, :])
            cvb = sm.tile([1, D], BF16, tag="cvb", name="cvb", bufs=1)
            nc.gpsimd.dma_start(out=cvb[:], in_=convbr[:1, :])
            ps_e = {}
            for t in range(NT):
                for js in range(2):
                    ps_e[(t, js)] = pp.tile([128, 512], F32, tag="ps", name="ps")
            for dc in range(DC):
                xt = tr.tile([128, 256], BF16, tag="xt", name="xt", bufs=3)
                nc.sync.dma_start(out=xt[:, 0:N],
                                  in_=xcolT[dc * 128:(dc + 1) * 128, :])
                cwt = wp.tile([128, 3072], BF16, tag="w", name="w")
                nc.sync.dma_start(out=cwt[:, 0:384],
                                  in_=convw[dc * 128:(dc + 1) * 128, 0:384])
                nc.sync.dma_start(out=cwt[:, 384:768],
                                  in_=convw[dc * 128:(dc + 1) * 128, 384:768])
                for t in range(NT):
                    for js in range(2):
                        nc.tensor.matmul(
                            ps_e[(t, js)][:, 0:384],
                            xt[:, t * 128:(t + 1) * 128],
                            cwt[:, js * 384:(js + 1) * 384],
                            start=(dc == 0), stop=False)
            # layer-0 attention weights ahead of the late consts: the sync
            # HWDGE ring drains FIFO, and layer 0 needs wqkvo first
            emit_wqkvo_dmas(0)
            late_const_dmas()
            patches = [tr.tile([128, D], F32, tag="t", name="t") for _ in range(NT)]
            for t in range(NT):
                for js in range(2):
                    # + conv_b via K=1 ones-row matmul (exact)
                    nc.tensor.matmul(
                        ps_e[(t, js)][:, 0:384], ones_row[:1, :],
                        cvb[:1, js * 384:(js + 1) * 384],
                        start=False, stop=True)
                    nc.vector.tensor_copy(
                        out=patches[t][:, js * 384:(js + 1) * 384],
                        in_=ps_e[(t, js)][:, 0:384])

            # GroupNorm stats over (group channels x all tokens)
            part = [st.tile([128, 2 * G], F32, tag="gnp", name="gnp")
                    for _ in range(NT)]
            for t in range(NT):
                sq = tr.tile([128, D], F32, tag="t", name="t")
                nc.scalar.activation(out=sq[:], in_=patches[t][:], func=AF.Square)
                for g in range(G):
                    nc.vector.reduce_sum(out=part[t][:, g:g + 1],
                                         in_=patches[t][:, g * GS:(g + 1) * GS],
                                         axis=mybir.AxisListType.X)
                    nc.vector.reduce_sum(out=part[t][:, G + g:G + g + 1],
                                         in_=sq[:, g * GS:(g + 1) * GS],
                                         axis=mybir.AxisListType.X)
            partb = [st.tile([128, 2 * G], BF16, tag="gnpb", name="gnpb")
                     for _ in range(NT)]
            for t in range(NT):
                nc.vector.tensor_copy(out=partb[t][:], in_=part[t][:])
            psg = pp.tile([128, 512], F32, tag="ps", name="ps")
            for t in range(NT):
                nc.tensor.matmul(psg[0:1, 0:2 * G], ones_col[:], partb[t][:],
                                 start=(t == 0), stop=(t == NT - 1))
            # gr: [0:768] gn_g, [768:1536] gn_b, [1536:2304] scratch row,
            #     [2304:2320] group stats
            inv_cnt = 1.0 / (GS * N)
            nc.vector.tensor_scalar_mul(out=gr[:, 2304:2304 + 2 * G],
                                        in0=psg[0:1, 0:2 * G], scalar1=inv_cnt)
            mg = gr[:, 2304:2304 + G]
            msq = gr[:, 2304 + G:2304 + 2 * G]
            mg2 = gr[:, 1536:1536 + G]
            nc.vector.tensor_mul(out=mg2, in0=mg, in1=mg)
            nc.vector.tensor_sub(out=msq, in0=msq, in1=mg2)
            nc.scalar.activation(out=msq, in_=msq, func=AF.Sqrt,
                                 bias=eps5[0:1, :])
            nc.vector.reciprocal(out=msq, in_=msq)
            # A = rstd_g * gn_g ; Bc = gn_b - mean_g * A (per-group scalars,
            # expanded across each group's 96 channels via stride-0 APs)
            rsx = ec.tile([1, D], F32, tag="gscr", name="gscr")
            grv = gr[:]

            def _gexp(col):
                return bass.AP(tensor=grv.tensor, offset=grv.offset + col,
                               ap=[grv.ap[0], [1, G], [0, GS]])

            arow = gr[:, 1536:2304]
            nc.vector.tensor_tensor(out=arow, in0=gr[:, 0:D],
                                    in1=_gexp(2304 + G), op=OP.mult)
            nc.vector.tensor_tensor(out=rsx[:, 0:D], in0=arow,
                                    in1=_gexp(2304), op=OP.mult)
            nc.vector.tensor_sub(out=rsx[:, 0:D], in0=gr[:, D:2 * D],
                                 in1=rsx[:, 0:D])
            ab = lc.tile([128, 2 * D], F32, tag="gnab", name="gnab", bufs=1)
            nc.gpsimd.partition_broadcast(ab[:, 0:D], arow)
            nc.gpsimd.partition_broadcast(ab[:, D:2 * D], rsx[:1, 0:D])
            for t in range(NT):
                tmp = tr.tile([128, D], F32, tag="t", name="t")
                nc.vector.tensor_mul(out=tmp[:], in0=patches[t][:], in1=ab[:, 0:D])
                nc.vector.tensor_add(out=h[t][:], in0=tmp[:], in1=ab[:, D:2 * D])

        # ================= transformer layers =================
        # per-layer const rows: small DMAs on the scalar ring, prefetched one
        # layer ahead so the issue slot isn't stuck behind a whole layer of
        # ACT-queue work
        consts = [None] * DEPTH

        def emit_const_dmas(j):
            pj = Lw[j]
            lcb1 = lc.tile([128, 3 * D], BF16, tag="lcb1", name="lcb1")
            nc.gpsimd.dma_start(out=lcb1[:],
                                in_=_row_bcast(pj["lrow"][:1, 0:3 * D], 3 * D))
            lcb2 = lc.tile([128, D], BF16, tag="lcb2", name="lcb2")
            nc.gpsimd.dma_start(
                out=lcb2[:],
                in_=bass.AP(tensor=pj["lrow"][:1, :].tensor,
                            offset=pj["lrow"][:1, :].offset + 3 * D,
                            ap=[[0, 128], [1, D]]))
            smalls = sm.tile([128, 12 + MC], F32, tag="sme", name="sme")
            nc.gpsimd.dma_start(out=smalls[:], in_=pj["smalls"][:, :])
            consts[j] = (lcb1, lcb2, smalls)

        emit_const_dmas(0)
        pend = None
        for i in range(DEPTH):
            p = Lw[i]
            with nc.named_scope(f"layer{i}"):
                lcb1, lcb2, smalls = consts[i]
                SHIFT = lcb1[:, 0:D]
                MOD1 = lcb1[:, D:2 * D]
                BV = lcb1[:, 2 * D:3 * D]
                B2 = lcb2[:, 0:D]
                BQP = smalls[:, 0:6]     # permuted q bias, col dc = chunk
                BKP = smalls[:, 6:12]
                B1C = smalls[:, 12:12 + MC]

                # weight tiles (prefetchable large DMAs, sync ring)
                if wqkvos[i] is None:
                    emit_wqkvo_dmas(i)
                wqkvo = wqkvos[i]

                # --- finalize h from previous layer's MLP + AdaLN + LN1 ---
                def finalize_h(t):
                    if pend is not None:
                        pps2, ph1B2 = pend
                        for js in range(2):
                            sl = slice(js * 384, (js + 1) * 384)
                            nc.vector.tensor_add(out=h[t][:, sl],
                                                 in0=pps2[(t, js)][:, 0:384],
                                                 in1=ph1B2[t][:, sl])

                hmod = [res.tile([128, D], F32, tag="res", name="res")
                        for _ in range(NT)]
                hn = [tr.tile([128, D], BF16, tag="hn", name="hn", bufs=4)
                      for _ in range(NT)]

                def ada_chain(t):
                    tmp = tr.tile([128, D], F32, tag="t", name="t")
                    ln_apply(h[t][:], tmp[:])
                    tmp2 = tr.tile([128, D], F32, tag="t", name="t")
                    nc.vector.tensor_mul(out=tmp2[:], in0=tmp[:], in1=MOD1)
                    nc.vector.tensor_add(out=hmod[t][:], in0=tmp2[:], in1=SHIFT)
                    ln_apply(hmod[t][:], hn[t][:])

                hnT = wt.tile([128, DC * N], BF16, tag="wt", name="wt")

                def hnT_transposes(t, dst, src):
                    transpose4([src[t][:, dc * 128:(dc + 1) * 128]
                                for dc in range(4)],
                               _ap3(dst[:], t * 128, 4, 256, 128), on_act=True)
                    transpose4([src[t][:, dc * 128:(dc + 1) * 128]
                                for dc in range(4, 6)],
                               _ap3(dst[:], 4 * 256 + t * 128, 2, 256, 128),
                               on_act=True)

                ps_v = {}

                def v_gemm(t):
                    for js in range(2):
                        ps_v[(t, js)] = pp.tile([128, 512], F32, tag="ps",
                                                name="ps")
                    for dc in range(DC):
                        for js in range(2):
                            nc.tensor.matmul(
                                ps_v[(t, js)][:, 0:384],
                                hnT[:, dc * N + t * 128:dc * N + (t + 1) * 128],
                                wqkvo[dc][:, 2 * D + js * 384:2 * D + (js + 1) * 384],
                                start=(dc == 0), stop=(dc == DC - 1))

                def v_aug_add(t):
                    for js in range(2):
                        nc.vector.tensor_tensor(
                            out=_ap3(v_aug[t][:], js * 6 * 66, 6, 66, 64),
                            in0=_ap3(ps_v[(t, js)][:, 0:384], 0, 6, 64, 64),
                            in1=_ap3(BV, js * 384, 6, 64, 64), op=OP.add)

                # packed Q|K PSUM per qc: cols [0:256]=Q, [256:512]=K,
                # each accumulated per token tile t (so tile t0's GEMMs can
                # run while tile t1's AdaLN chain is still on the DVE).
                psqk = [None] * DC

                def qk_gemm(t, qcs):
                    for qc in qcs:
                        if psqk[qc] is None:
                            psqk[qc] = pp.tile([128, 512], F32, tag="ps",
                                               name="ps")
                        for coff, kbase in ((0, 0), (D, 256)):
                            for dc in range(DC):
                                nc.tensor.matmul(
                                    psqk[qc][:, kbase + t * 128:
                                             kbase + (t + 1) * 128],
                                    wqkvo[dc][:, coff + qc * 128:
                                              coff + (qc + 1) * 128],
                                    hnT[:, dc * N + t * 128:
                                        dc * N + (t + 1) * 128],
                                    start=(dc == 0), stop=(dc == DC - 1))

                rotqk = [None] * DC

                def rot_chunk(qc):
                    """q|k rotary for one 128-chunk, 512-wide merged ops."""
                    qb = rt.tile([128, 512], BF16, tag="qb", name="qb")
                    nc.scalar.activation(out=qb[:, 0:256],
                                         in_=psqk[qc][:, 0:256],
                                         func=AF.Identity,
                                         bias=BQP[:, qc:qc + 1])
                    nc.scalar.activation(out=qb[:, 256:512],
                                         in_=psqk[qc][:, 256:512],
                                         func=AF.Identity,
                                         bias=BKP[:, qc:qc + 1])
                    sh = rt.tile([128, 512], BF16, tag="qb", name="qb")
                    nc.vector.stream_shuffle(sh[:], qb[:], SWAP_MASK)
                    u = rt.tile([128, 512], BF16, tag="qb", name="qb")
                    nc.vector.tensor_mul(out=u[:], in0=qb[:], in1=cosPt[:])
                    nc.vector.tensor_mul(out=sh[:], in0=sh[:], in1=sinPt[:])
                    rotqk[qc] = rq.tile([128, 512], BF16, tag="rq", name="rq")
                    nc.vector.tensor_add(out=rotqk[qc][:], in0=u[:], in1=sh[:])

                # ---- boundary + QKV, t-pipelined ----
                finalize_h(0)
                ada_chain(0)
                hnT_transposes(0, hnT, hn)
                v_gemm(0)
                qk_gemm(0, range(0, 4))
                finalize_h(1)
                ada_chain(1)
                # anchored dummy exp: pulls the exp-set ACT table load to
                # right after ada t1 (hidden under the QK GEMMs); the anchor
                # input stops the scheduler from hoisting it
                nc.scalar.activation(out=dmy[:1, 0:1], in_=hn[1][0:1, 0:1],
                                     func=AF.Exp)
                qk_gemm(0, range(4, DC))
                v_aug_add(0)
                hnT_transposes(1, hnT, hn)
                qk_gemm(1, [0])
                rot_chunk(0)
                v_gemm(1)
                v_aug_add(1)
                for qc in range(1, DC):
                    qk_gemm(1, [qc])
                    rot_chunk(qc)

                # ---- attention ----
                attn = [tr.tile([128, D], BF16, tag="at", name="at", bufs=2)
                        for _ in range(NT)]
                attnT = wt.tile([128, DC * N], BF16, tag="wt", name="wt")
                ps_o = {}

                def scores(hd_):
                    jc = hd_ // 2
                    po = (hd_ % 2) * 64
                    ps = pp.tile([128, 512], F32, tag="ps", name="ps")
                    for mc in range(NT):
                        nc.tensor.matmul(
                            ps[:, mc * 256:(mc + 1) * 256],
                            rotqk[jc][po:po + 64,
                                      256 + mc * 128:256 + (mc + 1) * 128],
                            rotqk[jc][po:po + 64, 0:256],
                            start=True, stop=True)
                    return ps

                pa = {}
                ps_s = scores(0)
                for hd_ in range(NH):
                    half = hd_ // 6
                    hi = hd_ % 6
                    if hi == 0:
                        for t in range(NT):
                            pa[(half, t)] = pp.tile([128, 512], F32, tag="ps",
                                                    name="ps")
                    es = ex.tile([128, 512], BF16, tag="ex", name="ex")
                    nc.scalar.activation(out=es[:], in_=ps_s[:, 0:512],
                                         func=AF.Exp, scale=HD ** -0.5)
                    if hd_ + 1 < NH:
                        ps_s = scores(hd_ + 1)
                    for t in range(NT):
                        for mc in range(NT):
                            nc.tensor.matmul(
                                pa[(half, t)][:, hi * 66:(hi + 1) * 66],
                                es[:, mc * 256 + t * 128:mc * 256 + (t + 1) * 128],
                                v_aug[mc][:, hd_ * 66:(hd_ + 1) * 66],
                                start=(mc == 0), stop=(mc == NT - 1))
                    if hi == 5:
                        # batched softmax normalize for this 6-head group
                        for t in range(NT):
                            pav = pa[(half, t)][:]
                            rz = st.tile([128, 8], F32, tag="rz", name="rz")
                            nc.vector.reciprocal(
                                out=rz[:, 0:6],
                                in_=bass.AP(tensor=pav.tensor,
                                            offset=pav.offset + 64,
                                            ap=[pav.ap[0], [66, 6], [1, 1]]))
                            rzv = rz[:]
                            nc.vector.tensor_tensor(
                                out=_ap3(attn[t][:], half * 384, 6, 64, 64),
                                in0=_ap3(pav, 0, 6, 66, 64),
                                in1=bass.AP(tensor=rzv.tensor,
                                            offset=rzv.offset,
                                            ap=[rzv.ap[0], [1, 6], [0, 64]]),
                                op=OP.mult)
                        for jc in range(3 * half, 3 * half + 3):
                            transpose4(
                                [attn[t][:, jc * 128:(jc + 1) * 128]
                                 for t in range(NT)],
                                attnT[:, jc * N:(jc + 1) * N])
                        if half == 0:
                            for js in range(2):
                                for t in range(NT):
                                    ps_o[(t, js)] = pp.tile([128, 512], F32,
                                                            tag="ps", name="ps")
                            for dc in range(0, 3):
                                for t in range(NT):
                                    for js in range(2):
                                        nc.tensor.matmul(
                                            ps_o[(t, js)][:, 0:384],
                                            attnT[:, dc * N + t * 128:
                                                  dc * N + (t + 1) * 128],
                                            wqkvo[dc][:, 3 * D + js * 384:
                                                      3 * D + (js + 1) * 384],
                                            start=(dc == 0), stop=False)
                        else:
                            # t0 finishes first (incl. folded bo bias) so its
                            # LN2 chain overlaps t1's remaining GEMMs
                            for t in range(NT):
                                for dc in range(3, DC):
                                    for js in range(2):
                                        nc.tensor.matmul(
                                            ps_o[(t, js)][:, 0:384],
                                            attnT[:, dc * N + t * 128:
                                                  dc * N + (t + 1) * 128],
                                            wqkvo[dc][:, 3 * D + js * 384:
                                                      3 * D + (js + 1) * 384],
                                            start=False, stop=(dc == DC - 1))

                # ---- MLP, t-pipelined ----
                w1l = []
                for dc in range(DC):
                    w_ = wp.tile([128, 3072], BF16, tag="w", name="w")
                    nc.sync.dma_start(out=w_[:],
                                      in_=p["w1"][dc * 128:(dc + 1) * 128, :])
                    w1l.append(w_)
                w2l = []
                for k in range(DC):
                    w_ = wp.tile([128, 3072], BF16, tag="w", name="w")
                    nc.sync.dma_start(out=w_[:],
                                      in_=p["w2p"][:, k * 3072:(k + 1) * 3072])
                    w2l.append(w_)

                h1 = [res.tile([128, D], F32, tag="res", name="res")
                      for _ in range(NT)]
                h1B2 = [res.tile([128, D], F32, tag="res", name="res")
                        for _ in range(NT)]
                hn2 = [tr.tile([128, D], BF16, tag="hn", name="hn", bufs=4)
                       for _ in range(NT)]
                hn2T = wt.tile([128, DC * N], BF16, tag="wt", name="wt")

                def ln2_chain(t):
                    for js in range(2):
                        sl = slice(js * 384, (js + 1) * 384)
                        nc.vector.tensor_add(out=h1[t][:, sl],
                                             in0=ps_o[(t, js)][:, 0:384],
                                             in1=hmod[t][:, sl])
                    ln_apply(h1[t][:], hn2[t][:])

                def hn2T_transposes(t):
                    transpose4([hn2[t][:, dc * 128:(dc + 1) * 128]
                                for dc in range(4)],
                               _ap3(hn2T[:], t * 128, 4, 256, 128))
                    transpose4([hn2[t][:, dc * 128:(dc + 1) * 128]
                                for dc in range(4, 6)],
                               _ap3(hn2T[:], 4 * 256 + t * 128, 2, 256, 128))

                ps1p = {}
                g_l = [None] * MC

                def mlp_up(mc, t):
                    k = mc // 2
                    if mc % 2 == 0 and t == 0:
                        ps1p[k] = pp.tile([128, 512], F32, tag="ps", name="ps")
                    base = (mc % 2) * 256 + t * 128
                    for dc in range(DC):
                        nc.tensor.matmul(
                            ps1p[k][:, base:base + 128],
                            w1l[dc][:, mc * 128:(mc + 1) * 128],
                            hn2T[:, dc * N + t * 128:dc * N + (t + 1) * 128],
                            start=(dc == 0), stop=(dc == DC - 1))

                def mlp_gelu(mc):
                    g_ = ge.tile([128, 256], BF16, tag="ge", name="ge")
                    nc.scalar.activation(
                        out=g_[:], in_=ps1p[mc // 2][:, (mc % 2) * 256:
                                                     (mc % 2) * 256 + 256],
                        func=AF.Gelu, bias=B1C[:, mc:mc + 1])
                    g_l[mc] = g_

                ps2 = {}

                def mlp_down(mc, t):
                    for js in range(2):
                        if mc == 0:
                            ps2[(t, js)] = pp.tile([128, 512], F32, tag="ps",
                                                   name="ps")
                        nc.tensor.matmul(
                            ps2[(t, js)][:, 0:384],
                            g_l[mc][:, t * 128:(t + 1) * 128],
                            w2l[mc // 4][:, (mc % 4) * D + js * 384:
                                         (mc % 4) * D + (js + 1) * 384],
                            start=(mc == 0), stop=(mc == MC - 1))

                ln2_chain(0)
                hn2T_transposes(0)
                for mc in range(U0):
                    mlp_up(mc, 0)
                ln2_chain(1)
                # anchored dummy: pull the gelu ACT table load to right after
                # ln2 t1, hidden under the up GEMMs
                nc.scalar.activation(out=dmy[:1, 0:1], in_=hn2[1][0:1, 0:1],
                                     func=AF.Gelu)
                hn2T_transposes(1)
                if i + 1 < DEPTH:
                    emit_const_dmas(i + 1)
                for mc in range(MC):
                    mlp_up(mc, 1)
                    mlp_gelu(mc)
                    if mc + U0 < MC:
                        mlp_up(mc + U0, 0)
                    if mc >= 1:
                        mlp_down(mc - 1, 0)
                mlp_down(MC - 1, 0)
                # h1B2 on the (idle) GPSIMD engine so the scheduler cannot
                # slot these ahead of the critical ln2 DVE ops
                for t in range(NT):
                    nc.gpsimd.tensor_tensor(out=h1B2[t][:], in0=h1[t][:],
                                            in1=B2, op=OP.add)
                for mc in range(MC):
                    mlp_down(mc, 1)
                pend = (ps2, h1B2)

        # ================= final layer =================
        with nc.named_scope("final"):
            ob = res.tile([128, D], F32, tag="res", name="ob")
            nc.gpsimd.dma_start(out=ob[:], in_=_row_bcast(outrow[:1, :], D))
            hf = [tr.tile([128, D], BF16, tag="hn", name="hn", bufs=4)
                  for _ in range(NT)]
            fps2, fh1B2 = pend
            owl = []
            for dc in range(DC):
                w_ = wp.tile([128, 3072], BF16, tag="w", name="w")
                nc.sync.dma_start(out=w_[:, 0:D],
                                  in_=outw[dc * 128:(dc + 1) * 128, :])
                owl.append(w_)
            hfT = wt.tile([128, DC * N], BF16, tag="wt", name="wt")
            ps_f = {}

            def fin_chain(t):
                for js in range(2):
                    sl = slice(js * 384, (js + 1) * 384)
                    nc.vector.tensor_add(out=h[t][:, sl],
                                         in0=fps2[(t, js)][:, 0:384],
                                         in1=fh1B2[t][:, sl])
                ln_apply(h[t][:], hf[t][:])

            def fin_transposes(t):
                transpose4([hf[t][:, dc * 128:(dc + 1) * 128]
                            for dc in range(4)],
                           _ap3(hfT[:], t * 128, 4, 256, 128), on_act=True)
                transpose4([hf[t][:, dc * 128:(dc + 1) * 128]
                            for dc in range(4, 6)],
                           _ap3(hfT[:], 4 * 256 + t * 128, 2, 256, 128),
                           on_act=True)

            def fin_gemm(t):
                for js in range(2):
                    ps_f[(t, js)] = pp.tile([128, 512], F32, tag="ps", name="ps")
                for dc in range(DC):
                    for js in range(2):
                        nc.tensor.matmul(
                            ps_f[(t, js)][:, 0:384],
                            hfT[:, dc * N + t * 128:dc * N + (t + 1) * 128],
                            owl[dc][:, js * 384:(js + 1) * 384],
                            start=(dc == 0), stop=(dc == DC - 1))

            def fin_out(t):
                osb = tr.tile([128, D], F32, tag="t", name="t")
                for js in range(2):
                    sl = slice(js * 384, (js + 1) * 384)
                    nc.vector.tensor_add(out=osb[:, sl],
                                         in0=ps_f[(t, js)][:, 0:384],
                                         in1=ob[:, sl])
                nc.sync.dma_start(out=out[t * 128:(t + 1) * 128, :], in_=osb[:])

            fin_chain(0)
            fin_transposes(0)
            fin_gemm(0)
            fin_chain(1)
            fin_transposes(1)
            fin_gemm(1)
            fin_out(0)
            fin_out(1)


# ---------------------------------------------------------------- host side

def _host_prep(inputs):
    f32 = np.float32
    x = np.asarray(inputs["x"], f32)
    t = np.asarray(inputs["t"], f32)

    # time embedding + AdaLN modulation (sidecar, ~0.25% of model FLOPs)
    ts = t * 1000.0
    half = 384
    freqs = np.exp(np.arange(half, dtype=f32) * f32(-math.log(10000.0) / (half - 1)))
    e = ts[:, None] * freqs[None, :]
    temb = np.concatenate([np.sin(e), np.cos(e)], axis=-1).astype(f32)
    a = temb @ np.asarray(inputs["t_w1"], f32) + np.asarray(inputs["t_b1"], f32)
    a = (a / (1.0 + np.exp(-a))).astype(f32)  # silu
    temb = (a @ np.asarray(inputs["t_w2"], f32)
            + np.asarray(inputs["t_b2"], f32)).astype(f32)
    stemb = (temb / (1.0 + np.exp(-temb))).astype(f32)  # silu(temb)
    ada_w = np.asarray(inputs["ada_w"], f32)
    ada_b = np.asarray(inputs["ada_b"], f32)
    sc = np.einsum("bk,iko->bio", stemb, ada_w).astype(f32) + ada_b[None]
    shift = sc[:, :, :D]
    mod1 = (1.0 + sc[:, :, D:]).astype(f32)

    # im2col (transposed): xcolT[b] [(c p q), n]
    xr = x.reshape(B, C_IN, HH // P, P, WW // P, P)
    xcol = xr.transpose(0, 2, 4, 1, 3, 5).reshape(B, N, D)
    xcolT = np.ascontiguousarray(xcol.transpose(0, 2, 1))

    convw = np.ascontiguousarray(np.asarray(inputs["conv_w"], f32).reshape(D, D).T)
    convbr = np.asarray(inputs["conv_b"], f32)[None]

    grow = np.zeros((1, 3 * D + 2 * G), f32)
    grow[0, 0:D] = np.asarray(inputs["gn_g"], f32)
    grow[0, D:2 * D] = np.asarray(inputs["gn_b"], f32)

    # rotary pair-interleaved permutation: within each head's 64 dims,
    # output order is [0, 32, 1, 33, ..., 31, 63]
    perm64 = np.empty(64, np.int64)
    perm64[0::2] = np.arange(32)
    perm64[1::2] = np.arange(32, 64)
    permD = np.concatenate([hh * 64 + perm64 for hh in range(NH)])

    # rotary tables in permuted transposed layout [128, N] (head pair)
    inv = (10000.0 ** (-(np.arange(0, HD, 2, dtype=f32)) / HD)).astype(f32)
    f_ = np.arange(N, dtype=f32)[:, None] * inv[None, :]  # [N, 32]
    cos_t = np.cos(f_).astype(f32)   # [N, 32]
    sin_t = np.sin(f_).astype(f32)
    cosP = np.empty((128, N), f32)
    sinP = np.empty((128, N), f32)
    for pidx in range(64):
        i_ = pidx // 2
        cosP[pidx] = cos_t[:, i_]
        sinP[pidx] = sin_t[:, i_] * (-1.0 if pidx % 2 == 0 else 1.0)
    cosP[64:] = cosP[:64]
    sinP[64:] = sinP[:64]
    cosPP = np.concatenate([cosP, cosP], axis=1)  # [128, 2N]: q|k merged
    sinPP = np.concatenate([sinP, sinP], axis=1)

    ln1_g = np.asarray(inputs["ln1_g"], f32)
    ln1_b = np.asarray(inputs["ln1_b"], f32)
    ln2_g = np.asarray(inputs["ln2_g"], f32)
    ln2_b = np.asarray(inputs["ln2_b"], f32)

    layers = []
    for i in range(DEPTH):
        wq = np.asarray(inputs["wq"][i], f32)
        wk = np.asarray(inputs["wk"][i], f32)
        wv = np.asarray(inputs["wv"][i], f32)
        wo = np.asarray(inputs["wo"][i], f32)
        g1 = ln1_g[i][:, None]
        bq = np.asarray(inputs["bq"][i], f32) + ln1_b[i] @ wq
        bk = np.asarray(inputs["bk"][i], f32) + ln1_b[i] @ wk
        bv = np.asarray(inputs["bv"][i], f32) + ln1_b[i] @ wv
        # permute q/k output columns for pair-interleaved rotary
        wqp = (g1 * wq)[:, permD]
        wkp = (g1 * wk)[:, permD]
        bqp = bq[permD]
        bkp = bk[permD]
        wqkvo = np.concatenate([wqp, wkp, g1 * wv, wo], axis=1).astype(f32)
        w1 = np.asarray(inputs["w1"][i], f32)
        w2 = np.asarray(inputs["w2"][i], f32)
        # w2 pre-chunked: [128, 24*768], block mc = w2[mc*128:(mc+1)*128, :]
        w2p = np.ascontiguousarray(
            w2.reshape(MC, 128, D).transpose(1, 0, 2).reshape(128, MC * D))
        b1 = (np.asarray(inputs["b1"][i], f32) + ln2_b[i] @ w1).astype(f32)
        smalls = np.zeros((128, 12 + MC), f32)
        smalls[:, 0:6] = bqp.reshape(6, 128).T
        smalls[:, 6:12] = bkp.reshape(6, 128).T
        smalls[:, 12:12 + MC] = b1.reshape(MC, 128).T
        bo = np.asarray(inputs["bo"][i], f32)
        b2 = np.asarray(inputs["b2"][i], f32)
        lrow = np.concatenate([
            np.zeros(D, f32), np.zeros(D, f32),  # shift, mod1 filled per-batch
            bv, b2]).astype(f32)[None]
        layers.append(dict(
            wqkvo=np.ascontiguousarray(wqkvo),
            w1=np.ascontiguousarray((ln2_g[i][:, None] * w1).astype(f32)),
            w2p=w2p,
            lrow=lrow,
            bo=bo,
            smalls=smalls,
        ))

    out_w = np.asarray(inputs["out_w"], f32)
    outw = np.ascontiguousarray(
        (np.asarray(inputs["fin_g"], f32)[:, None] * out_w).astype(f32))
    outrow = (np.asarray(inputs["out_b"], f32)
              + np.asarray(inputs["fin_b"], f32) @ out_w).astype(f32)[None]

    import ml_dtypes
    bfc = lambda a: np.ascontiguousarray(a.astype(ml_dtypes.bfloat16))
    in_maps = []
    for b in range(B):
        m = dict(
            xcolT=bfc(xcolT[b]),
            identm=bfc(np.eye(128, dtype=f32)),
            onesr=bfc(np.ones((1, 128), f32)),
            convw=bfc(convw), convbr=bfc(convbr), grow=grow,
            cosPP=bfc(cosPP), sinPP=bfc(sinPP), outw=bfc(outw), outrow=outrow,
        )
        for i, L in enumerate(layers):
            m[f"wqkvo{i}"] = bfc(L["wqkvo"])
            m[f"w1{i}"] = bfc(L["w1"])
            m[f"w2p{i}"] = bfc(L["w2p"])
            lr = L["lrow"].copy()
            lr[0, 0:D] = shift[b, i] + L["bo"]
            lr[0, D:2 * D] = mod1[b, i]
            m[f"lrow{i}"] = bfc(lr)
            m[f"smalls{i}"] = L["smalls"]
        in_maps.append(m)
    return in_maps


def kernel(**inputs):
    if "nc" not in _CACHE:
        _CACHE["nc"] = _build()
    nc = _CACHE["nc"]
    in_maps = _host_prep(inputs)
    trace = bool(os.environ.get("KERNEL_TRACE"))
    res = run_bass_kernel_spmd(nc, in_maps, list(range(B)), trace=trace)
    LAST_RESULT["res"] = res
    out = np.empty((B, C_IN, HH, WW), np.float32)
    for b in range(B):
        o = res.results[b]["out"]  # [256, 768] = [n, (c p q)]
        out[b] = (o.reshape(16, 16, C_IN, P, P)
                  .transpose(2, 0, 3, 1, 4).reshape(C_IN, HH, WW))
    return out


if __name__ == "__main__":
    _build()
    print("build ok")


# revision 28
# speedup vs baseline: 1.2189x; 1.0059x over previous
"""Trainium2 Bass kernel for nn_DiT_4758823763997 (DiT dense transformer).

B=8 batch, N=256 tokens, D=768, 12 layers, 12 heads (hd 64), MLP 3072.
Sharding: pure data-parallel - one batch element per NeuronCore (8 cores),
weights replicated; no collectives.

v3 design (vs v2):
  - Per-token-tile (t0/t1) software pipelining across ALL phase boundaries:
    the serial DVE LayerNorm/AdaLN chains for tile t run while the PE works
    on the other tile's GEMMs (prev-layer mlp-down t1, per-t QK, per-t
    mlp-up), eliminating the two ~8us PE stalls per layer.
  - LN rstd computed as exp(-0.5*ln(var+eps)): Ln and Exp share one ACT
    table set, so the only ACT table swaps left are gelu<->ln/exp (2 per
    layer), both prefetched via dummy ops while ACT is idle.
  - Rotary processed 512-wide (q|k merged per chunk) on DVE in bf16.
  - Softmax normalizer batched per 6-head group: AV results accumulate in
    one PSUM bank per (t, half), one strided reciprocal + one stride-0
    broadcast multiply replace 24 reciprocal+scale pairs.
  - Out-proj bias folded into the GEMM accumulation (ones-row matmul).
  - Small latency-critical DMAs (per-layer rows, GN row) issued on the
    scalar HWDGE ring so the weight-prefetch flood on the sync ring cannot
    delay them.
"""

import math
import os
import sys

sys.path.insert(0, "/opt/trn_rl_repo")

import numpy as np

import concourse.bass as bass
import concourse.bacc as bacc
import concourse.mybir as mybir
import concourse.tile as tile
from concourse.bass_utils import run_bass_kernel_spmd

B = 8
C_IN = 3
HH = 256
WW = 256
P = 16
D = 768
DEPTH = 12
NH = 12
HD = 64
MLPD = 3072
N = 256
G = 8
GS = D // G

F32 = mybir.dt.float32
BF16 = mybir.dt.bfloat16
AF = mybir.ActivationFunctionType
OP = mybir.AluOpType

DC = D // 128    # 6
NT = N // 128    # 2
MC = MLPD // 128  # 24
U0 = 6           # mlp-up t0 runahead chunks (even; bounds live PSUM pairs)

LAST_RESULT = {}
_CACHE = {}

# stream_shuffle mask: swap adjacent partitions within each 32-quadrant
SWAP_MASK = [i ^ 1 for i in range(32)]


def _ap3(ap2d, base, nblk, stride, width):
    """[128, nblk, width] free-strided view of a 2D AP at column offset base."""
    return bass.AP(tensor=ap2d.tensor, offset=ap2d.offset + base,
                   ap=[ap2d.ap[0], [stride, nblk], [1, width]])


def _row_bcast(row_ap, width, parts=128):
    """[1, W] row -> step-0 partition-broadcast AP [parts, W]."""
    return bass.AP(tensor=row_ap.tensor, offset=row_ap.offset,
                   ap=[[0, parts], [1, width]])


def _build():
    nc = bacc.Bacc("TRN2", target_bir_lowering=False, debug=False, num_devices=8)

    def din(name, shape, dt=BF16):
        return nc.declare_dram_parameter(name, list(shape), dt, isOutput=False)

    xcolT = din("xcolT", [D, N])
    identm = din("identm", [128, 128])
    onesr = din("onesr", [1, 128])
    convw = din("convw", [D, D])
    convbr = din("convbr", [1, D])
    grow = din("grow", [1, 3 * D + 2 * G], F32)   # gn_g | gn_b | scratch
    cosPP = din("cosPP", [128, 2 * N])
    sinPP = din("sinPP", [128, 2 * N])
    Lw = []
    for i in range(DEPTH):
        Lw.append(dict(
            wqkvo=din(f"wqkvo{i}", [D, 4 * D]),        # wq|wk|wv|wo (q,k col-permuted)
            w1=din(f"w1{i}", [D, MLPD]),
            w2p=din(f"w2p{i}", [128, MC * D]),          # pre-chunked [128, 24*768]
            lrow=din(f"lrow{i}", [1, 4 * D]),           # shift+bo|mod1|bv|b2 (bf16)
            smalls=din(f"smalls{i}", [128, 12 + MC], F32),  # bqP|bkP|b1c
        ))
    outw = din("outw", [D, D])
    outrow = din("outrow", [1, D], F32)
    out = nc.declare_dram_parameter("out", [N, D], F32, isOutput=True)

    with tile.TileContext(nc) as tc:
        _emit(nc, tc, xcolT, identm, onesr, convw, convbr, grow, cosPP, sinPP,
              Lw, outw, outrow, out)
    nc.compile()
    return nc


def _emit(nc, tc, xcolT, identm, onesr, convw, convbr, grow, cosPP, sinPP,
          Lw, outw, outrow, out):
    from contextlib import ExitStack
    with ExitStack() as ctx:
        pers = ctx.enter_context(tc.tile_pool(name="pers", bufs=1))
        wp = ctx.enter_context(tc.tile_pool(name="wp", bufs=13))     # [128,3072] bf16 weight tiles
        res = ctx.enter_context(tc.tile_pool(name="res", bufs=6))
        tr = ctx.enter_context(tc.tile_pool(name="tr", bufs=4))
        wt = ctx.enter_context(tc.tile_pool(name="wt", bufs=4))      # transposed activations bf16
        rq = ctx.enter_context(tc.tile_pool(name="rq", bufs=7))      # rotated q|k chunks
        rt = ctx.enter_context(tc.tile_pool(name="rt", bufs=4))      # rotary transients
        st = ctx.enter_context(tc.tile_pool(name="st", bufs=6))
        ex = ctx.enter_context(tc.tile_pool(name="ex", bufs=3))
        ge = ctx.enter_context(tc.tile_pool(name="ge", bufs=26))     # gelu chunks (24 live)
        lc = ctx.enter_context(tc.tile_pool(name="lc", bufs=2))      # bcast rows bf16
        sm = ctx.enter_context(tc.tile_pool(name="sm", bufs=2))      # smalls
        ec = ctx.enter_context(tc.tile_pool(name="ec", bufs=1))
        pp = ctx.enter_context(tc.tile_pool(name="pp", bufs=8, space="PSUM"))

        ident = pers.tile([128, 128], BF16, tag="ident", name="ident")
        ones_col = pers.tile([128, 1], BF16, tag="onesc", name="onesc")
        nc.sync.dma_start(
            out=ones_col[:],
            in_=bass.AP(tensor=onesr[:1, :].tensor, offset=onesr[:1, :].offset,
                        ap=[[0, 128], [1, 1]]))
        ones_row = pers.tile([1, 128], BF16, tag="onesr", name="onesr")
        nc.sync.dma_start(out=ones_row[:], in_=onesr[:1, :])
        eps6 = pers.tile([128, 1], F32, tag="eps6", name="eps6")
        nc.vector.memset(eps6[:], 1e-6)
        eps5 = pers.tile([128, 1], F32, tag="eps5", name="eps5")
        nc.vector.memset(eps5[:], 1e-5)
        dmy = pers.tile([1, 2], F32, tag="dmy", name="dmy")
        nc.vector.memset(dmy[:], 1.0)

        cosPt = pers.tile([128, 2 * N], BF16, tag="cosPP", name="cosPP")
        sinPt = pers.tile([128, 2 * N], BF16, tag="sinPP", name="sinPP")

        h = [pers.tile([128, D], F32, tag=f"h{t}", name=f"h{t}") for t in range(NT)]
        v_aug = [pers.tile([128, NH * 66], BF16, tag=f"va{t}", name=f"va{t}")
                 for t in range(NT)]

        def late_const_dmas():
            # constants not needed until layer 0: emitted after the embed's
            # input DMAs so they don't delay the first conv matmuls
            nc.sync.dma_start(out=ident[:], in_=identm[:, :])
            nc.sync.dma_start(out=cosPt[:], in_=cosPP[:, :])
            nc.sync.dma_start(out=sinPt[:], in_=sinPP[:, :])
            for t in range(NT):
                va = v_aug[t][:]
                nc.sync.dma_start(
                    out=bass.AP(tensor=va.tensor, offset=va.offset + 64,
                                ap=[va.ap[0], [66, NH], [1, 2]]),
                    in_=bass.AP(tensor=onesr[:1, :].tensor,
                                offset=onesr[:1, :].offset,
                                ap=[[0, 128], [1, 2 * NH]]))

        def ln_apply(x_ap, out_ap):
            """out = (x - mean)/sqrt(var + 1e-6) along free dim 768."""
            s = st.tile([128, 16], F32, tag="lnst", name="lnst")
            nc.vector.bn_stats(out=s[:, 0:6], in_=x_ap[:, 0:384])
            nc.vector.bn_stats(out=s[:, 6:12], in_=x_ap[:, 384:768])
            sv = s[:]
            nc.vector.bn_aggr(
                out=s[:, 12:14],
                in_=bass.AP(tensor=sv.tensor, offset=sv.offset,
                            ap=[sv.ap[0], [6, 2], [1, 6]]))
            nc.scalar.activation(out=s[:, 14:15], in_=s[:, 13:14],
                                 func=AF.Sqrt, bias=eps6[:])
            nc.vector.reciprocal(out=s[:, 14:15], in_=s[:, 14:15])
            nc.vector.tensor_scalar(
                out=out_ap, in0=x_ap, scalar1=s[:, 12:13], scalar2=s[:, 14:15],
                op0=OP.subtract, op1=OP.mult)

        def transpose4(srcs, dst_ap, on_act=False):
            """Transpose up to 4 [128,128] bf16 blocks via PE into one PSUM
            tile, then one copy into dst_ap ([128, 128*len] bf16) on DVE or
            (when DVE is the contended engine) the ACT engine."""
            ps = pp.tile([128, 512], BF16, tag="ps", name="pst")
            for k, s_ in enumerate(srcs):
                nc.tensor.transpose(ps[:, k * 128:(k + 1) * 128], s_, ident[:])
            if on_act:
                nc.scalar.activation(out=dst_ap, in_=ps[:, 0:128 * len(srcs)],
                                     func=AF.Identity)
            else:
                nc.vector.tensor_copy(out=dst_ap, in_=ps[:, 0:128 * len(srcs)])

        wqkvos = [None] * DEPTH

        def emit_wqkvo_dmas(j):
            lst = []
            for dc in range(DC):
                w_ = wp.tile([128, 3072], BF16, tag="w", name="w")
                nc.sync.dma_start(out=w_[:],
                                  in_=Lw[j]["wqkvo"][dc * 128:(dc + 1) * 128, :])
                lst.append(w_)
            wqkvos[j] = lst

        # ================= patch embed =================
        with nc.named_scope("embed"):
            # tiny latency-critical row: scalar HWDGE ring, ahead of the
            # weight-prefetch flood on the sync ring
            gr = ec.tile([1, 3 * D + 2 * G], F32, tag="grows", name="grows")
            nc.gpsimd.dma_start(out=gr[:], in_=grow[:1, :])
            cvb = sm.tile([1, D], BF16, tag="cvb", name="cvb", bufs=1)
            nc.gpsimd.dma_start(out=cvb[:], in_=convbr[:1, :])
            ps_e = {}
            for t in range(NT):
                for js in range(2):
                    ps_e[(t, js)] = pp.tile([128, 512], F32, tag="ps", name="ps")
            for dc in range(DC):
                xt = tr.tile([128, 256], BF16, tag="xt", name="xt", bufs=3)
                nc.sync.dma_start(out=xt[:, 0:N],
                                  in_=xcolT[dc * 128:(dc + 1) * 128, :])
                cwt = wp.tile([128, 3072], BF16, tag="w", name="w")
                nc.sync.dma_start(out=cwt[:, 0:384],
                                  in_=convw[dc * 128:(dc + 1) * 128, 0:384])
                nc.sync.dma_start(out=cwt[:, 384:768],
                                  in_=convw[dc * 128:(dc + 1) * 128, 384:768])
                for t in range(NT):
                    for js in range(2):
                        nc.tensor.matmul(
                            ps_e[(t, js)][:, 0:384],
                            xt[:, t * 128:(t + 1) * 128],
                            cwt[:, js * 384:(js + 1) * 384],
                            start=(dc == 0), stop=False)
            # layer-0 attention weights ahead of the late consts: the sync
            # HWDGE ring drains FIFO, and layer 0 needs wqkvo first
            emit_wqkvo_dmas(0)
            late_const_dmas()
            patches = [tr.tile([128, D], F32, tag="t", name="t") for _ in range(NT)]
            for t in range(NT):
                for js in range(2):
                    # + conv_b via K=1 ones-row matmul (exact)
                    nc.tensor.matmul(
                        ps_e[(t, js)][:, 0:384], ones_row[:1, :],
                        cvb[:1, js * 384:(js + 1) * 384],
                        start=False, stop=True)
                    nc.vector.tensor_copy(
                        out=patches[t][:, js * 384:(js + 1) * 384],
                        in_=ps_e[(t, js)][:, 0:384])

            # GroupNorm stats over (group channels x all tokens)
            part = [st.tile([128, 2 * G], F32, tag="gnp", name="gnp")
                    for _ in range(NT)]
            for t in range(NT):
                sq = tr.tile([128, D], F32, tag="t", name="t")
                nc.scalar.activation(out=sq[:], in_=patches[t][:], func=AF.Square)
                for g in range(G):
                    nc.vector.reduce_sum(out=part[t][:, g:g + 1],
                                         in_=patches[t][:, g * GS:(g + 1) * GS],
                                         axis=mybir.AxisListType.X)
                    nc.vector.reduce_sum(out=part[t][:, G + g:G + g + 1],
                                         in_=sq[:, g * GS:(g + 1) * GS],
                                         axis=mybir.AxisListType.X)
            partb = [st.tile([128, 2 * G], BF16, tag="gnpb", name="gnpb")
                     for _ in range(NT)]
            for t in range(NT):
                nc.vector.tensor_copy(out=partb[t][:], in_=part[t][:])
            psg = pp.tile([128, 512], F32, tag="ps", name="ps")
            for t in range(NT):
                nc.tensor.matmul(psg[0:1, 0:2 * G], ones_col[:], partb[t][:],
                                 start=(t == 0), stop=(t == NT - 1))
            # gr: [0:768] gn_g, [768:1536] gn_b, [1536:2304] scratch row,
            #     [2304:2320] group stats
            inv_cnt = 1.0 / (GS * N)
            nc.vector.tensor_scalar_mul(out=gr[:, 2304:2304 + 2 * G],
                                        in0=psg[0:1, 0:2 * G], scalar1=inv_cnt)
            mg = gr[:, 2304:2304 + G]
            msq = gr[:, 2304 + G:2304 + 2 * G]
            mg2 = gr[:, 1536:1536 + G]
            nc.vector.tensor_mul(out=mg2, in0=mg, in1=mg)
            nc.vector.tensor_sub(out=msq, in0=msq, in1=mg2)
            nc.scalar.activation(out=msq, in_=msq, func=AF.Sqrt,
                                 bias=eps5[0:1, :])
            nc.vector.reciprocal(out=msq, in_=msq)
            # A = rstd_g * gn_g ; Bc = gn_b - mean_g * A (per-group scalars,
            # expanded across each group's 96 channels via stride-0 APs)
            rsx = ec.tile([1, D], F32, tag="gscr", name="gscr")
            grv = gr[:]

            def _gexp(col):
                return bass.AP(tensor=grv.tensor, offset=grv.offset + col,
                               ap=[grv.ap[0], [1, G], [0, GS]])

            arow = gr[:, 1536:2304]
            nc.vector.tensor_tensor(out=arow, in0=gr[:, 0:D],
                                    in1=_gexp(2304 + G), op=OP.mult)
            nc.vector.tensor_tensor(out=rsx[:, 0:D], in0=arow,
                                    in1=_gexp(2304), op=OP.mult)
            nc.vector.tensor_sub(out=rsx[:, 0:D], in0=gr[:, D:2 * D],
                                 in1=rsx[:, 0:D])
            ab = lc.tile([128, 2 * D], F32, tag="gnab", name="gnab", bufs=1)
            nc.gpsimd.partition_broadcast(ab[:, 0:D], arow)
            nc.gpsimd.partition_broadcast(ab[:, D:2 * D], rsx[:1, 0:D])
            # anchored PE fillers through the serial GN-row + first AdaLN
            # chain (PE would otherwise sit idle ~17us and cold-clock)
            psf = pp.tile([128, 512], F32, tag="ps", name="pst")
            for _ in range(30):
                nc.tensor.matmul(psf[0:16, 0:512], partb[0][:, 0:16],
                                 cosPt[:, 0:512], start=True, stop=True)
            for t in range(NT):
                tmp = tr.tile([128, D], F32, tag="t", name="t")
                nc.vector.tensor_mul(out=tmp[:], in0=patches[t][:], in1=ab[:, 0:D])
                nc.vector.tensor_add(out=h[t][:], in0=tmp[:], in1=ab[:, D:2 * D])

        # ================= transformer layers =================
        # per-layer const rows: small DMAs on the scalar ring, prefetched one
        # layer ahead so the issue slot isn't stuck behind a whole layer of
        # ACT-queue work
        consts = [None] * DEPTH

        def emit_const_dmas(j):
            pj = Lw[j]
            lcb1 = lc.tile([128, 3 * D], BF16, tag="lcb1", name="lcb1")
            nc.gpsimd.dma_start(out=lcb1[:],
                                in_=_row_bcast(pj["lrow"][:1, 0:3 * D], 3 * D))
            lcb2 = lc.tile([128, D], BF16, tag="lcb2", name="lcb2")
            nc.gpsimd.dma_start(
                out=lcb2[:],
                in_=bass.AP(tensor=pj["lrow"][:1, :].tensor,
                            offset=pj["lrow"][:1, :].offset + 3 * D,
                            ap=[[0, 128], [1, D]]))
            smalls = sm.tile([128, 12 + MC], F32, tag="sme", name="sme")
            nc.gpsimd.dma_start(out=smalls[:], in_=pj["smalls"][:, :])
            consts[j] = (lcb1, lcb2, smalls)

        emit_const_dmas(0)
        pend = None
        for i in range(DEPTH):
            p = Lw[i]
            with nc.named_scope(f"layer{i}"):
                lcb1, lcb2, smalls = consts[i]
                SHIFT = lcb1[:, 0:D]
                MOD1 = lcb1[:, D:2 * D]
                BV = lcb1[:, 2 * D:3 * D]
                B2 = lcb2[:, 0:D]
                BQP = smalls[:, 0:6]     # permuted q bias, col dc = chunk
                BKP = smalls[:, 6:12]
                B1C = smalls[:, 12:12 + MC]

                # weight tiles (prefetchable large DMAs, sync ring)
                if wqkvos[i] is None:
                    emit_wqkvo_dmas(i)
                wqkvo = wqkvos[i]

                # --- finalize h from previous layer's MLP + AdaLN + LN1 ---
                def finalize_h(t):
                    if pend is not None:
                        pps2, ph1B2 = pend
                        for js in range(2):
                            sl = slice(js * 384, (js + 1) * 384)
                            nc.vector.tensor_add(out=h[t][:, sl],
                                                 in0=pps2[(t, js)][:, 0:384],
                                                 in1=ph1B2[t][:, sl])

                hmod = [res.tile([128, D], F32, tag="res", name="res")
                        for _ in range(NT)]
                hn = [tr.tile([128, D], BF16, tag="hn", name="hn", bufs=4)
                      for _ in range(NT)]

                def ada_chain(t):
                    tmp = tr.tile([128, D], F32, tag="t", name="t")
                    ln_apply(h[t][:], tmp[:])
                    tmp2 = tr.tile([128, D], F32, tag="t", name="t")
                    nc.vector.tensor_mul(out=tmp2[:], in0=tmp[:], in1=MOD1)
                    nc.vector.tensor_add(out=hmod[t][:], in0=tmp2[:], in1=SHIFT)
                    ln_apply(hmod[t][:], hn[t][:])

                hnT = wt.tile([128, DC * N], BF16, tag="wt", name="wt")

                def hnT_transposes(t, dst, src):
                    transpose4([src[t][:, dc * 128:(dc + 1) * 128]
                                for dc in range(4)],
                               _ap3(dst[:], t * 128, 4, 256, 128), on_act=True)
                    transpose4([src[t][:, dc * 128:(dc + 1) * 128]
                                for dc in range(4, 6)],
                               _ap3(dst[:], 4 * 256 + t * 128, 2, 256, 128),
                               on_act=True)

                ps_v = {}

                def v_gemm(t):
                    for js in range(2):
                        ps_v[(t, js)] = pp.tile([128, 512], F32, tag="ps",
                                                name="ps")
                    for dc in range(DC):
                        for js in range(2):
                            nc.tensor.matmul(
                                ps_v[(t, js)][:, 0:384],
                                hnT[:, dc * N + t * 128:dc * N + (t + 1) * 128],
                                wqkvo[dc][:, 2 * D + js * 384:2 * D + (js + 1) * 384],
                                start=(dc == 0), stop=(dc == DC - 1))

                def v_aug_add(t):
                    for js in range(2):
                        nc.vector.tensor_tensor(
                            out=_ap3(v_aug[t][:], js * 6 * 66, 6, 66, 64),
                            in0=_ap3(ps_v[(t, js)][:, 0:384], 0, 6, 64, 64),
                            in1=_ap3(BV, js * 384, 6, 64, 64), op=OP.add)

                # packed Q|K PSUM per qc: cols [0:256]=Q, [256:512]=K,
                # each accumulated per token tile t (so tile t0's GEMMs can
                # run while tile t1's AdaLN chain is still on the DVE).
                psqk = [None] * DC

                def qk_gemm(t, qcs):
                    for qc in qcs:
                        if psqk[qc] is None:
                            psqk[qc] = pp.tile([128, 512], F32, tag="ps",
                                               name="ps")
                        for coff, kbase in ((0, 0), (D, 256)):
                            for dc in range(DC):
                                nc.tensor.matmul(
                                    psqk[qc][:, kbase + t * 128:
                                             kbase + (t + 1) * 128],
                                    wqkvo[dc][:, coff + qc * 128:
                                              coff + (qc + 1) * 128],
                                    hnT[:, dc * N + t * 128:
                                        dc * N + (t + 1) * 128],
                                    start=(dc == 0), stop=(dc == DC - 1))

                rotqk = [None] * DC

                def rot_chunk(qc):
                    """q|k rotary for one 128-chunk, 512-wide merged ops."""
                    qb = rt.tile([128, 512], BF16, tag="qb", name="qb")
                    nc.scalar.activation(out=qb[:, 0:256],
                                         in_=psqk[qc][:, 0:256],
                                         func=AF.Identity,
                                         bias=BQP[:, qc:qc + 1])
                    nc.scalar.activation(out=qb[:, 256:512],
                                         in_=psqk[qc][:, 256:512],
                                         func=AF.Identity,
                                         bias=BKP[:, qc:qc + 1])
                    sh = rt.tile([128, 512], BF16, tag="qb", name="qb")
                    nc.vector.stream_shuffle(sh[:], qb[:], SWAP_MASK)
                    u = rt.tile([128, 512], BF16, tag="qb", name="qb")
                    nc.vector.tensor_mul(out=u[:], in0=qb[:], in1=cosPt[:])
                    nc.vector.tensor_mul(out=sh[:], in0=sh[:], in1=sinPt[:])
                    rotqk[qc] = rq.tile([128, 512], BF16, tag="rq", name="rq")
                    nc.vector.tensor_add(out=rotqk[qc][:], in0=u[:], in1=sh[:])

                # ---- boundary + QKV, t-pipelined ----
                finalize_h(0)
                ada_chain(0)
                hnT_transposes(0, hnT, hn)
                v_gemm(0)
                qk_gemm(0, range(0, 4))
                finalize_h(1)
                ada_chain(1)
                # anchored dummy exp: pulls the exp-set ACT table load to
                # right after ada t1 (hidden under the QK GEMMs); the anchor
                # input stops the scheduler from hoisting it
                nc.scalar.activation(out=dmy[:1, 0:1], in_=hn[1][0:1, 0:1],
                                     func=AF.Exp)
                qk_gemm(0, range(4, DC))
                v_aug_add(0)
                hnT_transposes(1, hnT, hn)
                qk_gemm(1, [0])
                rot_chunk(0)
                v_gemm(1)
                v_aug_add(1)
                for qc in range(1, DC):
                    qk_gemm(1, [qc])
                    rot_chunk(qc)

                # ---- attention ----
                attn = [tr.tile([128, D], BF16, tag="at", name="at", bufs=2)
                        for _ in range(NT)]
                attnT = wt.tile([128, DC * N], BF16, tag="wt", name="wt")
                ps_o = {}

                def scores(hd_):
                    jc = hd_ // 2
                    po = (hd_ % 2) * 64
                    ps = pp.tile([128, 512], F32, tag="ps", name="ps")
                    for mc in range(NT):
                        nc.tensor.matmul(
                            ps[:, mc * 256:(mc + 1) * 256],
                            rotqk[jc][po:po + 64,
                                      256 + mc * 128:256 + (mc + 1) * 128],
                            rotqk[jc][po:po + 64, 0:256],
                            start=True, stop=True)
                    return ps

                pa = {}
                ps_s = scores(0)
                for hd_ in range(NH):
                    half = hd_ // 6
                    hi = hd_ % 6
                    if hi == 0:
                        for t in range(NT):
                            pa[(half, t)] = pp.tile([128, 512], F32, tag="ps",
                                                    name="ps")
                    es = ex.tile([128, 512], BF16, tag="ex", name="ex")
                    nc.scalar.activation(out=es[:], in_=ps_s[:, 0:512],
                                         func=AF.Exp, scale=HD ** -0.5)
                    if hd_ + 1 < NH:
                        ps_s = scores(hd_ + 1)
                    for t in range(NT):
                        for mc in range(NT):
                            nc.tensor.matmul(
                                pa[(half, t)][:, hi * 66:(hi + 1) * 66],
                                es[:, mc * 256 + t * 128:mc * 256 + (t + 1) * 128],
                                v_aug[mc][:, hd_ * 66:(hd_ + 1) * 66],
                                start=(mc == 0), stop=(mc == NT - 1))
                    if hi == 5:
                        # batched softmax normalize for this 6-head group
                        for t in range(NT):
                            pav = pa[(half, t)][:]
                            rz = st.tile([128, 8], F32, tag="rz", name="rz")
                            nc.vector.reciprocal(
                                out=rz[:, 0:6],
                                in_=bass.AP(tensor=pav.tensor,
                                            offset=pav.offset + 64,
                                            ap=[pav.ap[0], [66, 6], [1, 1]]))
                            rzv = rz[:]
                            nc.vector.tensor_tensor(
                                out=_ap3(attn[t][:], half * 384, 6, 64, 64),
                                in0=_ap3(pav, 0, 6, 66, 64),
                                in1=bass.AP(tensor=rzv.tensor,
                                            offset=rzv.offset,
                                            ap=[rzv.ap[0], [1, 6], [0, 64]]),
                                op=OP.mult)
                        for jc in range(3 * half, 3 * half + 3):
                            transpose4(
                                [attn[t][:, jc * 128:(jc + 1) * 128]
                                 for t in range(NT)],
                                attnT[:, jc * N:(jc + 1) * N])
                        if half == 0:
                            for js in range(2):
                                for t in range(NT):
                                    ps_o[(t, js)] = pp.tile([128, 512], F32,
                                                            tag="ps", name="ps")
                            for dc in range(0, 3):
                                for t in range(NT):
                                    for js in range(2):
                                        nc.tensor.matmul(
                                            ps_o[(t, js)][:, 0:384],
                                            attnT[:, dc * N + t * 128:
                                                  dc * N + (t + 1) * 128],
                                            wqkvo[dc][:, 3 * D + js * 384:
                                                      3 * D + (js + 1) * 384],
                                            start=(dc == 0), stop=False)
                        else:
                            # t0 finishes first (incl. folded bo bias) so its
                            # LN2 chain overlaps t1's remaining GEMMs
                            for t in range(NT):
                                for dc in range(3, DC):
                                    for js in range(2):
                                        nc.tensor.matmul(
                                            ps_o[(t, js)][:, 0:384],
                                            attnT[:, dc * N + t * 128:
                                                  dc * N + (t + 1) * 128],
                                            wqkvo[dc][:, 3 * D + js * 384:
                                                      3 * D + (js + 1) * 384],
                                            start=False, stop=(dc == DC - 1))

                # ---- MLP, t-pipelined ----
                w1l = []
                for dc in range(DC):
                    w_ = wp.tile([128, 3072], BF16, tag="w", name="w")
                    nc.sync.dma_start(out=w_[:],
                                      in_=p["w1"][dc * 128:(dc + 1) * 128, :])
                    w1l.append(w_)
                w2l = []
                for k in range(DC):
                    w_ = wp.tile([128, 3072], BF16, tag="w", name="w")
                    nc.sync.dma_start(out=w_[:],
                                      in_=p["w2p"][:, k * 3072:(k + 1) * 3072])
                    w2l.append(w_)

                h1 = [res.tile([128, D], F32, tag="res", name="res")
                      for _ in range(NT)]
                h1B2 = [res.tile([128, D], F32, tag="res", name="res")
                        for _ in range(NT)]
                hn2 = [tr.tile([128, D], BF16, tag="hn", name="hn", bufs=4)
                       for _ in range(NT)]
                hn2T = wt.tile([128, DC * N], BF16, tag="wt", name="wt")

                def ln2_chain(t):
                    for js in range(2):
                        sl = slice(js * 384, (js + 1) * 384)
                        nc.vector.tensor_add(out=h1[t][:, sl],
                                             in0=ps_o[(t, js)][:, 0:384],
                                             in1=hmod[t][:, sl])
                    ln_apply(h1[t][:], hn2[t][:])

                def hn2T_transposes(t):
                    transpose4([hn2[t][:, dc * 128:(dc + 1) * 128]
                                for dc in range(4)],
                               _ap3(hn2T[:], t * 128, 4, 256, 128))
                    transpose4([hn2[t][:, dc * 128:(dc + 1) * 128]
                                for dc in range(4, 6)],
                               _ap3(hn2T[:], 4 * 256 + t * 128, 2, 256, 128))

                ps1p = {}
                g_l = [None] * MC

                def mlp_up(mc, t):
                    k = mc // 2
                    if mc % 2 == 0 and t == 0:
                        ps1p[k] = pp.tile([128, 512], F32, tag="ps", name="ps")
                    base = (mc % 2) * 256 + t * 128
                    for dc in range(DC):
                        nc.tensor.matmul(
                            ps1p[k][:, base:base + 128],
                            w1l[dc][:, mc * 128:(mc + 1) * 128],
                            hn2T[:, dc * N + t * 128:dc * N + (t + 1) * 128],
                            start=(dc == 0), stop=(dc == DC - 1))

                def mlp_gelu(mc):
                    g_ = ge.tile([128, 256], BF16, tag="ge", name="ge")
                    nc.scalar.activation(
                        out=g_[:], in_=ps1p[mc // 2][:, (mc % 2) * 256:
                                                     (mc % 2) * 256 + 256],
                        func=AF.Gelu, bias=B1C[:, mc:mc + 1])
                    g_l[mc] = g_

                ps2 = {}

                def mlp_down(mc, t):
                    for js in range(2):
                        if mc == 0:
                            ps2[(t, js)] = pp.tile([128, 512], F32, tag="ps",
                                                   name="ps")
                        nc.tensor.matmul(
                            ps2[(t, js)][:, 0:384],
                            g_l[mc][:, t * 128:(t + 1) * 128],
                            w2l[mc // 4][:, (mc % 4) * D + js * 384:
                                         (mc % 4) * D + (js + 1) * 384],
                            start=(mc == 0), stop=(mc == MC - 1))

                ln2_chain(0)
                # anchored PE fillers: keep the HAM clock warm while the LN2
                # chains run; anchored on attn (written by the half-1 scale)
                # so the scheduler cannot hoist them
                psf = pp.tile([128, 512], F32, tag="ps", name="pst")
                for _ in range(10):
                    nc.tensor.matmul(psf[:, 0:512], attn[0][:, 0:128],
                                     attn[0][:, 0:512], start=True, stop=True)
                hn2T_transposes(0)
                for mc in range(U0):
                    mlp_up(mc, 0)
                ln2_chain(1)
                # anchored dummy: pull the gelu ACT table load to right after
                # ln2 t1, hidden under the up GEMMs
                nc.scalar.activation(out=dmy[:1, 0:1], in_=hn2[1][0:1, 0:1],
                                     func=AF.Gelu)
                hn2T_transposes(1)
                if i + 1 < DEPTH:
                    emit_const_dmas(i + 1)
                for mc in range(MC):
                    mlp_up(mc, 1)
                    mlp_gelu(mc)
                    if mc + U0 < MC:
                        mlp_up(mc + U0, 0)
                    if mc >= 1:
                        mlp_down(mc - 1, 0)
                mlp_down(MC - 1, 0)
                # h1B2 on the (idle) GPSIMD engine so the scheduler cannot
                # slot these ahead of the critical ln2 DVE ops
                for t in range(NT):
                    nc.gpsimd.tensor_tensor(out=h1B2[t][:], in0=h1[t][:],
                                            in1=B2, op=OP.add)
                for mc in range(MC):
                    mlp_down(mc, 1)
                pend = (ps2, h1B2)

        # ================= final layer =================
        with nc.named_scope("final"):
            ob = res.tile([128, D], F32, tag="res", name="ob")
            nc.gpsimd.dma_start(out=ob[:], in_=_row_bcast(outrow[:1, :], D))
            hf = [tr.tile([128, D], BF16, tag="hn", name="hn", bufs=4)
                  for _ in range(NT)]
            fps2, fh1B2 = pend
            owl = []
            for dc in range(DC):
                w_ = wp.tile([128, 3072], BF16, tag="w", name="w")
                nc.sync.dma_start(out=w_[:, 0:D],
                                  in_=outw[dc * 128:(dc + 1) * 128, :])
                owl.append(w_)
            hfT = wt.tile([128, DC * N], BF16, tag="wt", name="wt")
            ps_f = {}

            def fin_chain(t):
                for js in range(2):
                    sl = slice(js * 384, (js + 1) * 384)
                    nc.vector.tensor_add(out=h[t][:, sl],
                                         in0=fps2[(t, js)][:, 0:384],
                                         in1=fh1B2[t][:, sl])
                ln_apply(h[t][:], hf[t][:])

            def fin_transposes(t):
                transpose4([hf[t][:, dc * 128:(dc + 1) * 128]
                            for dc in range(4)],
                           _ap3(hfT[:], t * 128, 4, 256, 128), on_act=True)
                transpose4([hf[t][:, dc * 128:(dc + 1) * 128]
                            for dc in range(4, 6)],
                           _ap3(hfT[:], 4 * 256 + t * 128, 2, 256, 128),
                           on_act=True)

            def fin_gemm(t):
                for js in range(2):
                    ps_f[(t, js)] = pp.tile([128, 512], F32, tag="ps", name="ps")
                for dc in range(DC):
                    for js in range(2):
                        nc.tensor.matmul(
                            ps_f[(t, js)][:, 0:384],
                            hfT[:, dc * N + t * 128:dc * N + (t + 1) * 128],
                            owl[dc][:, js * 384:(js + 1) * 384],
                            start=(dc == 0), stop=(dc == DC - 1))

            def fin_out(t):
                osb = tr.tile([128, D], F32, tag="t", name="t")
                for js in range(2):
                    sl = slice(js * 384, (js + 1) * 384)
                    nc.vector.tensor_add(out=osb[:, sl],
                                         in0=ps_f[(t, js)][:, 0:384],
                                         in1=ob[:, sl])
                nc.sync.dma_start(out=out[t * 128:(t + 1) * 128, :], in_=osb[:])

            fin_chain(0)
            fin_transposes(0)
            fin_gemm(0)
            fin_chain(1)
            fin_transposes(1)
            fin_gemm(1)
            fin_out(0)
            fin_out(1)


# ---------------------------------------------------------------- host side

def _host_prep(inputs):
    f32 = np.float32
    x = np.asarray(inputs["x"], f32)
    t = np.asarray(inputs["t"], f32)

    # time embedding + AdaLN modulation (sidecar, ~0.25% of model FLOPs)
    ts = t * 1000.0
    half = 384
    freqs = np.exp(np.arange(half, dtype=f32) * f32(-math.log(10000.0) / (half - 1)))
    e = ts[:, None] * freqs[None, :]
    temb = np.concatenate([np.sin(e), np.cos(e)], axis=-1).astype(f32)
    a = temb @ np.asarray(inputs["t_w1"], f32) + np.asarray(inputs["t_b1"], f32)
    a = (a / (1.0 + np.exp(-a))).astype(f32)  # silu
    temb = (a @ np.asarray(inputs["t_w2"], f32)
            + np.asarray(inputs["t_b2"], f32)).astype(f32)
    stemb = (temb / (1.0 + np.exp(-temb))).astype(f32)  # silu(temb)
    ada_w = np.asarray(inputs["ada_w"], f32)
    ada_b = np.asarray(inputs["ada_b"], f32)
    sc = np.einsum("bk,iko->bio", stemb, ada_w).astype(f32) + ada_b[None]
    shift = sc[:, :, :D]
    mod1 = (1.0 + sc[:, :, D:]).astype(f32)

    # im2col (transposed): xcolT[b] [(c p q), n]
    xr = x.reshape(B, C_IN, HH // P, P, WW // P, P)
    xcol = xr.transpose(0, 2, 4, 1, 3, 5).reshape(B, N, D)
    xcolT = np.ascontiguousarray(xcol.transpose(0, 2, 1))

    convw = np.ascontiguousarray(np.asarray(inputs["conv_w"], f32).reshape(D, D).T)
    convbr = np.asarray(inputs["conv_b"], f32)[None]

    grow = np.zeros((1, 3 * D + 2 * G), f32)
    grow[0, 0:D] = np.asarray(inputs["gn_g"], f32)
    grow[0, D:2 * D] = np.asarray(inputs["gn_b"], f32)

    # rotary pair-interleaved permutation: within each head's 64 dims,
    # output order is [0, 32, 1, 33, ..., 31, 63]
    perm64 = np.empty(64, np.int64)
    perm64[0::2] = np.arange(32)
    perm64[1::2] = np.arange(32, 64)
    permD = np.concatenate([hh * 64 + perm64 for hh in range(NH)])

    # rotary tables in permuted transposed layout [128, N] (head pair)
    inv = (10000.0 ** (-(np.arange(0, HD, 2, dtype=f32)) / HD)).astype(f32)
    f_ = np.arange(N, dtype=f32)[:, None] * inv[None, :]  # [N, 32]
    cos_t = np.cos(f_).astype(f32)   # [N, 32]
    sin_t = np.sin(f_).astype(f32)
    cosP = np.empty((128, N), f32)
    sinP = np.empty((128, N), f32)
    for pidx in range(64):
        i_ = pidx // 2
        cosP[pidx] = cos_t[:, i_]
        sinP[pidx] = sin_t[:, i_] * (-1.0 if pidx % 2 == 0 else 1.0)
    cosP[64:] = cosP[:64]
    sinP[64:] = sinP[:64]
    cosPP = np.concatenate([cosP, cosP], axis=1)  # [128, 2N]: q|k merged
    sinPP = np.concatenate([sinP, sinP], axis=1)

    ln1_g = np.asarray(inputs["ln1_g"], f32)
    ln1_b = np.asarray(inputs["ln1_b"], f32)
    ln2_g = np.asarray(inputs["ln2_g"], f32)
    ln2_b = np.asarray(inputs["ln2_b"], f32)

    layers = []
    for i in range(DEPTH):
        wq = np.asarray(inputs["wq"][i], f32)
        wk = np.asarray(inputs["wk"][i], f32)
        wv = np.asarray(inputs["wv"][i], f32)
        wo = np.asarray(inputs["wo"][i], f32)
        g1 = ln1_g[i][:, None]
        bq = np.asarray(inputs["bq"][i], f32) + ln1_b[i] @ wq
        bk = np.asarray(inputs["bk"][i], f32) + ln1_b[i] @ wk
        bv = np.asarray(inputs["bv"][i], f32) + ln1_b[i] @ wv
        # permute q/k output columns for pair-interleaved rotary
        wqp = (g1 * wq)[:, permD]
        wkp = (g1 * wk)[:, permD]
        bqp = bq[permD]
        bkp = bk[permD]
        wqkvo = np.concatenate([wqp, wkp, g1 * wv, wo], axis=1).astype(f32)
        w1 = np.asarray(inputs["w1"][i], f32)
        w2 = np.asarray(inputs["w2"][i], f32)
        # w2 pre-chunked: [128, 24*768], block mc = w2[mc*128:(mc+1)*128, :]
        w2p = np.ascontiguousarray(
            w2.reshape(MC, 128, D).transpose(1, 0, 2).reshape(128, MC * D))
        b1 = (np.asarray(inputs["b1"][i], f32) + ln2_b[i] @ w1).astype(f32)
        smalls = np.zeros((128, 12 + MC), f32)
        smalls[:, 0:6] = bqp.reshape(6, 128).T
        smalls[:, 6:12] = bkp.reshape(6, 128).T
        smalls[:, 12:12 + MC] = b1.reshape(MC, 128).T
        bo = np.asarray(inputs["bo"][i], f32)
        b2 = np.asarray(inputs["b2"][i], f32)
        lrow = np.concatenate([
            np.zeros(D, f32), np.zeros(D, f32),  # shift, mod1 filled per-batch
            bv, b2]).astype(f32)[None]
        layers.append(dict(
            wqkvo=np.ascontiguousarray(wqkvo),
            w1=np.ascontiguousarray((ln2_g[i][:, None] * w1).astype(f32)),
            w2p=w2p,
            lrow=lrow,
            bo=bo,
            smalls=smalls,
        ))

    out_w = np.asarray(inputs["out_w"], f32)
    outw = np.ascontiguousarray(
        (np.asarray(inputs["fin_g"], f32)[:, None] * out_w).astype(f32))
    outrow = (np.asarray(inputs["out_b"], f32)
              + np.asarray(inputs["fin_b"], f32) @ out_w).astype(f32)[None]

    import ml_dtypes
    bfc = lambda a: np.ascontiguousarray(a.astype(ml_dtypes.bfloat16))
    in_maps = []
    for b in range(B):
        m = dict(
            xcolT=bfc(xcolT[b]),
            identm=bfc(np.eye(128, dtype=f32)),
            onesr=bfc(np.ones((1, 128), f32)),
            convw=bfc(convw), convbr=bfc(convbr), grow=grow,
            cosPP=bfc(cosPP), sinPP=bfc(sinPP), outw=bfc(outw), outrow=outrow,
        )
        for i, L in enumerate(layers):
            m[f"wqkvo{i}"] = bfc(L["wqkvo"])
            m[f"w1{i}"] = bfc(L["w1"])
            m[f"w2p{i}"] = bfc(L["w2p"])
            lr = L["lrow"].copy()
            lr[0, 0:D] = shift[b, i] + L["bo"]
            lr[0, D:2 * D] = mod1[b, i]
            m[f"lrow{i}"] = bfc(lr)
            m[f"smalls{i}"] = L["smalls"]
        in_maps.append(m)
    return in_maps


def kernel(**inputs):
    if "nc" not in _CACHE:
        _CACHE["nc"] = _build()
    nc = _CACHE["nc"]
    in_maps = _host_prep(inputs)
    trace = bool(os.environ.get("KERNEL_TRACE"))
    res = run_bass_kernel_spmd(nc, in_maps, list(range(B)), trace=trace)
    LAST_RESULT["res"] = res
    out = np.empty((B, C_IN, HH, WW), np.float32)
    for b in range(B):
        o = res.results[b]["out"]  # [256, 768] = [n, (c p q)]
        out[b] = (o.reshape(16, 16, C_IN, P, P)
                  .transpose(2, 0, 3, 1, 4).reshape(C_IN, HH, WW))
    return out


if __name__ == "__main__":
    _build()
    print("build ok")
